# revision 1
# baseline (speedup 1.0000x reference)
"""Trainium2 Bass kernel for nn_Attention (Gaussian banded attention).

Math (reference):
    v = values @ input_weights.T                      # [B,L,D]
    probs[h,q,k] = N(k - q - off_h; std_h)            # Gaussian, depends on k-q only
    attended[b,h,q,:] = sum_k probs[h,q,k] v[b,k,h*pd:(h+1)*pd]
    out = attended_merged @ output_weight.T           # [B,L,D]

Key structural facts exploited:
  - probs is a banded Toeplitz matrix per head: nonzero only for
    k - q in [off - 6*std, off + 6*std] (6-sigma truncation, error ~1e-8).
    Widest band: std=8, off=-8 -> k-q in [-56, 40].
  - So attention is a narrow depthwise convolution along L; no [L,L] matmul.
  - Batch x L sharding is embarrassingly parallel given a halo of
    56 backward / 40 forward rows of the INPUT (v is a row-wise projection,
    zero rows project to zero since there is no bias).

Sharding: 8 cores = (B=2) x (4 chunks of 512 rows of L). Each core gets
x.T zero-padded to [1024, 640] (56 halo + 512 + 40 halo + 32 zero pad),
computes in [D, L]->[L, D]->[D, L] layouts on the TensorEngine in bf16,
and writes out.T [1024, 512] bf16 (host casts back to f32 on reassembly).
No collectives.

Cost-model performance (CoreSim, TRN2 timing): 38,609 ns single execution;
33,046 ns/iter steady state = TensorE 100% busy (gapless streaming floor).
"""

import math
from contextlib import ExitStack

import numpy as np
import ml_dtypes

import concourse.bass as bass
from concourse import mybir
from concourse.bass_utils import run_bass_kernel_spmd

# ---- NEFF disk cache (keyed by BIR hash) to avoid recompiling identical
# graphs in fresh processes ----
import hashlib
import os
import shutil

_NEFF_CACHE_DIR = os.environ.get("NEFF_CACHE_DIR", "/root/neff_cache")


def _install_neff_cache():
    import concourse.bass_utils as _bu
    import concourse.bass2jax as _b2j
    if getattr(_bu, "_neff_cache_installed", False):
        return
    orig = _bu.compile_bir_kernel

    def cached(bir_json, tmpdir, neff_name="file.neff"):
        cpath = None
        try:
            os.makedirs(_NEFF_CACHE_DIR, exist_ok=True)
            key = hashlib.sha256(bir_json).hexdigest()[:32]
            cpath = os.path.join(_NEFF_CACHE_DIR, f"{key}.neff")
            dst = os.path.join(tmpdir, neff_name)
            if os.path.exists(cpath):
                shutil.copy(cpath, dst)
                return dst
        except OSError:
            cpath = None  # cache unusable; plain compile below
        path = orig(bir_json, tmpdir, neff_name)
        if cpath is not None:
            try:
                shutil.copy(path, cpath)
            except OSError:
                pass
        return path

    _bu.compile_bir_kernel = cached
    _b2j.compile_bir_kernel = cached
    _bu._neff_cache_installed = True


_install_neff_cache()

# ---------------- problem constants (hardcoded per spec) ----------------
B, L, D = 2, 2048, 1024
H, PD = 8, 128
ATTN_STD = np.array([1.0, 2.0, 4.0, 8.0, 1.0, 2.0, 4.0, 8.0], dtype=np.float64)
ATTN_OFFSET = np.array([-1.0, -2.0, -4.0, -8.0, -1.0, -2.0, -4.0, -8.0], dtype=np.float64)

N_CORES = 8
CHUNK = 512            # output rows per core
HALO_L, HALO_R = 56, 40
LPAD = 640             # 56 + 512 + 40 = 608, padded to 5*128
LT = 5                 # l-tiles of v (640 / 128)
KT = 8                 # d tiles (1024 / 128)
NQ = CHUNK             # query columns per core

BF16 = mybir.dt.bfloat16
F32 = mybir.dt.float32

G1 = LT * 2            # proj1 groups: (l-tile, n-chunk) -> v
G2 = H                 # attention heads -> attendedT
G3 = KT                # proj2 d_out tiles -> outT
NPS = 4                # rotating PSUM banks


def gauss_toeplitz_table() -> np.ndarray:
    """tp[h, r, m] = g_h(r - (m - 512) - 56), shape [H, 128, 1024] bf16.

    For v-tile t (rows k' = 128t + r of padded-local v) the attention rhs is
    tp[h][:, 512-128t : 1024-128t] so that rhs[r, q'] = g_h(128t + r - q' - 56),
    which is probs[h, q, k].T in padded-local coordinates.
    """
    r = np.arange(128, dtype=np.float64)[:, None]
    m = np.arange(1024, dtype=np.float64)[None, :]
    delta = r - (m - 512.0) - 56.0  # = k - q
    tables = []
    for h in range(H):
        std, off = ATTN_STD[h], ATTN_OFFSET[h]
        z = (delta - off) / std
        g = np.exp(-0.5 * z * z) / (std * math.sqrt(2.0 * math.pi))
        g[np.abs(z) > 6.0] = 0.0
        tables.append(g)
    return np.stack(tables).astype(ml_dtypes.bfloat16)


def attn_windows(h: int):
    """Static (t, j0, j1) list: nonzero q-column window of v-tile t for head h,
    8-aligned. Coverage of [0,512) is guaranteed (window width > 128)."""
    std, off = int(ATTN_STD[h]), int(ATTN_OFFSET[h])
    wlo = -56 - off - 6 * std
    whi = 71 - off + 6 * std
    res = []
    for t in range(LT):
        j0 = max(0, 128 * t + wlo)
        j1 = min(NQ, 128 * t + whi + 1)
        if j0 >= j1:
            continue
        j0 = (j0 // 8) * 8
        j1 = min(NQ, ((j1 + 7) // 8) * 8)
        res.append((t, j0, j1))
    return res


def build_graph(iters: int = 1, banded: bool = True) -> bass.Bass:
    """One SPMD core program. iters>1 repeats the whole kernel (including
    DMAs) with monotonically increasing semaphore thresholds, for timing.

    Phase structure per iteration (PE program order):
      warmup: 3x N=256 + 1x N=184 discarded matmuls on a zeroed tile during
              the first DMA's latency window (p-state ramp off the critical
              path; the last MM is sized to land just past data-readiness);
      wave A: v[:, 0:512]  = x @ W1a  -- k-outer over psum banks 0-4 so the
              PE streams while the xt/w1a DMAs arrive;
      wave B: v[:, 512:1024] = x @ W1b -- k-inner, data resident, banks [5,6,7,0,1];
      ph2:    attendedT per head, banded Toeplitz windows, banks [2,3,4,5];
      ph3:    outT = W2 @ attendedT, banks [0,1,6,7] (so the last output
              copies gate nothing until wave B of the NEXT iteration).
    Copies: wave A -> vector, wave B -> scalar, ph2/ph3 alternate engines;
    xt/w1 double-buffered so iterations pipeline with zero PE gaps.
    """
    nc = bass.Bass()

    xt = nc.declare_dram_parameter("xt", [D, LPAD], BF16, isOutput=False)
    w1t = nc.declare_dram_parameter("w1t", [D, D], BF16, isOutput=False)
    w2t = nc.declare_dram_parameter("w2t", [D, D], BF16, isOutput=False)
    tp = nc.declare_dram_parameter("tp", [H, 128, 1024], BF16, isOutput=False)
    out = nc.declare_dram_parameter("out", [D, NQ], BF16, isOutput=True)

    xt_r = xt[:].rearrange("(o p) f -> p o f", p=128)    # [128, 8, 640]
    w1_r = w1t[:].rearrange("(o p) f -> p o f", p=128)   # [128, 8, 1024]
    w2_r = w2t[:].rearrange("(o p) f -> p o f", p=128)   # [128, 8, 1024]
    tp_r = tp[:].rearrange("h p f -> p h f")             # [128, 8, 1024]

    with ExitStack() as ctx:
        e = ctx.enter_context
        xt_sb = e(nc.sbuf_tensor("xt_sb", [128, 2, KT, LPAD], BF16))
        w1_sb = e(nc.sbuf_tensor("w1_sb", [128, 2, KT, D], BF16))
        w2_sb = e(nc.sbuf_tensor("w2_sb", [128, KT, D], BF16))
        TP0, TPW = (408, 240) if banded else (0, 1024)
        tp_sb = e(nc.sbuf_tensor("tp_sb", [128, H, TPW], BF16))
        tp_src = tp_r[:, :, TP0:TP0 + TPW]
        v_sb = e(nc.sbuf_tensor("v_sb", [128, LT, D], BF16))
        at_sb = e(nc.sbuf_tensor("at_sb", [128, H, NQ], BF16))
        o_sb = e(nc.sbuf_tensor("o_sb", [128, KT, NQ], BF16))
        zdum = e(nc.sbuf_tensor("zdum", [128, 384], BF16))
        ps = [e(nc.psum_tensor(f"ps{i}", [128, 512], F32)) for i in range(8)]

        sem_names = (["zd", "mmA", "mm1", "mm2", "mm3", "tp_d",
                      "cpA", "cpB", "cp2v", "cp2s", "cp3v", "cp3s"]
                     + [f"xt_d{k}b{p}" for k in range(KT) for p in (0, 1)]
                     + [f"{n}b{p}" for n in ("w1a_d0", "w1a_g1", "w1a_g2",
                                             "w1b_g1", "w1b_g2") for p in (0, 1)]
                     + ["w2_g1", "w2_g2"]
                     + [f"dmo{m}" for m in range(G3)])
        sems = {n: e(nc.semaphore(n)) for n in sem_names}

        WAVE_B_BANKS = [5, 6, 7, 0, 1]
        PH2_BANKS = [2, 3, 4, 5]
        PH3_BANKS = [0, 1, 6, 7]

        def cp2_sem(h):
            return sems["cp2v" if h % 2 == 0 else "cp2s"]

        def cp2_count(h, it):
            return it * 4 + h // 2 + 1

        def cp3_waits(m, it):
            """(sem, count) pairs proving ph3 group m is fully copied out."""
            s = sems["cp3v" if m % 2 == 0 else "cp3s"]
            return [(s, it * 4 + m // 2 + 1)]

        def cp3_sem(m):
            return cp3_waits(m, 0)[0][0]

        def cp3_count(m, it):
            return cp3_waits(m, it)[0][1]

        with nc.Block() as block:

            @block.sync
            def _(sync: bass.BassEngine):
                for it in range(iters):
                    buf = it % 2
                    if it > 1:
                        # xt/w1 buffer reuse: wave B (last reader) of iter it-2
                        sync.wait_ge(sems["mm1"], (it - 1) * LT)
                    def xt_dma(k):
                        sync.dma_start(out=xt_sb[:, buf, k, :],
                                       in_=xt_r[:, k, :]).then_inc(
                            sems[f"xt_d{k}b{buf}"], 16)

                    # schedule tuned so the HWDGE generator (625ns/DMA, shared)
                    # stays ahead of wave A's per-k consumption
                    xt_dma(0)
                    sync.dma_start(out=w1_sb[:, buf, 0, 0:512],
                                   in_=w1_r[:, 0, 0:512]).then_inc(
                        sems[f"w1a_d0b{buf}"], 16)
                    xt_dma(1)
                    sync.dma_start(out=w1_sb[:, buf, 1:4, 0:512],
                                   in_=w1_r[:, 1:4, 0:512]).then_inc(
                        sems[f"w1a_g1b{buf}"], 16)
                    xt_dma(2)
                    xt_dma(3)
                    sync.dma_start(out=w1_sb[:, buf, 4:8, 0:512],
                                   in_=w1_r[:, 4:8, 0:512]).then_inc(
                        sems[f"w1a_g2b{buf}"], 16)
                    for k in range(4, KT):
                        xt_dma(k)
                    sync.dma_start(out=w1_sb[:, buf, 0:4, 512:1024],
                                   in_=w1_r[:, 0:4, 512:1024]).then_inc(
                        sems[f"w1b_g1b{buf}"], 16)
                    sync.dma_start(out=w1_sb[:, buf, 4:8, 512:1024],
                                   in_=w1_r[:, 4:8, 512:1024]).then_inc(
                        sems[f"w1b_g2b{buf}"], 16)
                    if it == 0:
                        sync.dma_start(out=tp_sb[:], in_=tp_src).then_inc(
                            sems["tp_d"], 16)
                    if it > 0:
                        sync.wait_ge(sems["mm3"], it * G3)
                    sync.dma_start(out=w2_sb[:, 0:4, :],
                                   in_=w2_r[:, 0:4, :]).then_inc(sems["w2_g1"], 16)
                    sync.dma_start(out=w2_sb[:, 4:8, :],
                                   in_=w2_r[:, 4:8, :]).then_inc(sems["w2_g2"], 16)


            @block.tensor
            def _(tensor: bass.BassEngine):
                # HAM/p-state warmup: discarded matmuls into bank 0 while the
                # first input DMAs are in flight (wave A k=0 start=True clears)
                tensor.wait_ge(sems["zd"], 1)
                for _ in range(3):
                    tensor.matmul(ps[0][:, 0:256], zdum[:, 0:128],
                                  zdum[:, 128:384], start=True, stop=True)
                # final warmup trimmed to N=184 so the PE arrives at the wave-A
                # wait cluster just after data-readiness (the cost model defers
                # dispatch by ~1.6us if the PE arrives early - measured cliff)
                tensor.matmul(ps[0][:, 0:184], zdum[:, 0:128],
                              zdum[:, 128:312], start=True, stop=True)
                for it in range(iters):
                    buf = it % 2
                    # ---- wave A: v[:, 0:512], k-outer, banks 0-4 ----
                    # cross-iter bank WAR: last users in iter it-1 were
                    # ph2 (banks 2,3,4 via h=4,5,6; bank 5 via h=7) and
                    # ph3 (banks 6,7,0,1 via m=4,5,6,7)
                    nth = (it // 2 + 1) * 16  # per-parity DMA count
                    for k in range(KT):
                        tensor.wait_ge(sems[f"xt_d{k}b{buf}"], nth)
                        if k == 0:
                            tensor.wait_ge(sems[f"w1a_d0b{buf}"], nth)
                        elif k == 1:
                            tensor.wait_ge(sems[f"w1a_g1b{buf}"], nth)
                        elif k == 4:
                            tensor.wait_ge(sems[f"w1a_g2b{buf}"], nth)
                        for lt in range(LT):
                            if k == 0 and it > 0:
                                if lt == 0:
                                    for s, c in cp3_waits(4, it - 1):
                                        tensor.wait_ge(s, c)
                                elif lt == 1:
                                    for s, c in cp3_waits(5, it - 1):
                                        tensor.wait_ge(s, c)
                                else:  # banks 2,3,4 <- ph2 h=4,5,6
                                    tensor.wait_ge(cp2_sem(lt + 2),
                                                   cp2_count(lt + 2, it - 1))
                            mm = tensor.matmul(
                                ps[lt][:, :],
                                xt_sb[:, buf, k, 128 * lt:128 * lt + 128],
                                w1_sb[:, buf, k, 0:512],
                                start=(k == 0), stop=(k == KT - 1),
                            )
                            if k == KT - 1:
                                mm.then_inc(sems["mmA"])
                    # ---- wave B: v[:, 512:1024], k-inner, banks [5,6,7,0,1] ----
                    for lt in range(LT):
                        bank = ps[WAVE_B_BANKS[lt]]
                        if lt == 0:
                            if it > 0:  # bank 5 <- ph2 h=7 of prev iter
                                tensor.wait_ge(cp2_sem(7), cp2_count(7, it - 1))
                        elif lt == 1:
                            if it > 0:  # bank 6 <- ph3 m=6 of prev iter
                                for s, c in cp3_waits(6, it - 1):
                                    tensor.wait_ge(s, c)
                        elif lt == 2:
                            if it > 0:  # bank 7 <- ph3 m=7 of prev iter
                                for s, c in cp3_waits(7, it - 1):
                                    tensor.wait_ge(s, c)
                        elif lt == 3:
                            # bank 0 <- wave A lt=0 copy of this iter
                            tensor.wait_ge(sems["cpA"], it * LT + 1)
                        else:
                            # bank 1 <- wave A lt=1 copy of this iter
                            tensor.wait_ge(sems["cpA"], it * LT + 2)
                        for k in range(KT):
                            if lt == 0 and k == 0:
                                tensor.wait_ge(sems[f"w1b_g1b{buf}"], nth)
                            elif lt == 0 and k == 4:
                                tensor.wait_ge(sems[f"w1b_g2b{buf}"], nth)
                            mm = tensor.matmul(
                                bank[:, :],
                                xt_sb[:, buf, k, 128 * lt:128 * lt + 128],
                                w1_sb[:, buf, k, 512:1024],
                                start=(k == 0), stop=(k == KT - 1),
                            )
                            if k == KT - 1:
                                mm.then_inc(sems["mm1"])

                    # ---- phase 2: attendedT per head (banded), banks 0-3 ----
                    if it == 0:
                        tensor.wait_ge(sems["tp_d"], 16)
                    for h in range(G2):
                        bank = ps[PH2_BANKS[h % 4]]
                        # bank WAR: banks 2,3,4 <- wave A lt=2,3,4 copies;
                        # bank 5 <- wave B g=0 copy; h>=4 <- ph2 head h-4
                        if h == 0:
                            tensor.wait_ge(sems["cpA"], it * LT + 3)
                        elif h == 1:
                            tensor.wait_ge(sems["cpA"], it * LT + 4)
                        elif h == 2:
                            tensor.wait_ge(sems["cpA"], it * LT + 5)
                        elif h == 3:
                            tensor.wait_ge(sems["cpB"], it * LT + 1)
                        else:
                            tensor.wait_ge(cp2_sem(h - 4), cp2_count(h - 4, it))
                        windows = attn_windows(h) if banded else [
                            (t, 0, NQ) for t in range(LT)]
                        for wi, (t, j0, j1) in enumerate(windows):
                            # data: v tile t, n-chunk h//4
                            if h // 4 == 0:
                                tensor.wait_ge(sems["cpA"], it * LT + t + 1)
                            else:
                                tensor.wait_ge(sems["cpB"], it * LT + t + 1)
                            c0 = 512 - 128 * t + j0 - TP0
                            c1 = 512 - 128 * t + j1 - TP0
                            mm = tensor.matmul(
                                bank[:, j0:j1],
                                v_sb[:, t, 128 * h:128 * h + 128],
                                tp_sb[:, h, c0:c1],
                                start=(wi == 0), stop=(wi == len(windows) - 1),
                            )
                            if wi == len(windows) - 1:
                                mm.then_inc(sems["mm2"])

                    # ---- phase 3: outT = W2 @ attendedT, banks 0-3 ----
                    for m in range(G3):
                        bank = ps[PH3_BANKS[m % 4]]
                        # bank WAR: banks 6,7,0,1 <- wave B g=1,2,3,4 copies;
                        # m>=4 <- ph3 copy m-4
                        if m < 4:
                            # banks [0,1,6,7] freed by wave B groups [3,4,1,2]
                            tensor.wait_ge(sems["cpB"],
                                           it * LT + [4, 5, 2, 3][m])
                        else:
                            for s, c in cp3_waits(m - 4, it):
                                tensor.wait_ge(s, c)
                        for k in range(KT):
                            if m == 0:
                                tensor.wait_ge(cp2_sem(k), cp2_count(k, it))
                                if k == 0:
                                    tensor.wait_ge(sems["w2_g1"], (it + 1) * 16)
                                elif k == 4:
                                    tensor.wait_ge(sems["w2_g2"], (it + 1) * 16)
                            mm = tensor.matmul(
                                bank[:, :],
                                w2_sb[:, k, 128 * m:128 * m + 128],
                                at_sb[:, k, :],
                                start=(k == 0), stop=(k == KT - 1),
                            )
                            if k == KT - 1:
                                mm.then_inc(sems["mm3"])

            @block.vector
            def _(vector: bass.BassEngine):
                for it in range(iters):
                    # wave A copies: v[:, lt, 0:512], banks 0-4, after k=7 MM
                    for lt in range(LT):
                        vector.wait_ge(sems["mmA"], it * LT + lt + 1)
                        vector.tensor_copy(
                            out=v_sb[:, lt, 0:512], in_=ps[lt][:, :],
                        ).then_inc(sems["cpA"])
                    for h in range(G2):
                        if h % 2 != 0:
                            continue
                        vector.wait_ge(sems["mm2"], it * G2 + h + 1)
                        vector.tensor_copy(
                            out=at_sb[:, h, :], in_=ps[PH2_BANKS[h % 4]][:, :],
                        ).then_inc(sems["cp2v"])
                    for m in [0, 2, 4, 6]:
                        vector.wait_ge(sems["mm3"], it * G3 + m + 1)
                        if it > 0:
                            vector.wait_ge(sems[f"dmo{m}"], it * 16)
                        vector.tensor_copy(
                            out=o_sb[:, m, :],
                            in_=ps[PH3_BANKS[m % 4]][:, :],
                        ).then_inc(sems["cp3v"])

            @block.gpsimd
            def _(gpsimd: bass.BassEngine):
                # zero the PE-warmup tile before anything else
                gpsimd.memset(zdum[:], 0).then_inc(sems["zd"])
                for it in range(iters):
                    for m in range(G3):
                        for s, c in cp3_waits(m, it):
                            gpsimd.wait_ge(s, c)
                        gpsimd.dma_start(
                            out=out[128 * m:128 * m + 128, :],
                            in_=o_sb[:, m, :],
                        ).then_inc(sems[f"dmo{m}"], 16)
                for m in range(G3):
                    gpsimd.wait_ge(sems[f"dmo{m}"], iters * 16)

            @block.scalar
            def _(scalar: bass.BassEngine):
                for it in range(iters):
                    # wave B copies: v[:, lt, 512:1024] from banks [4,5,6,7,4]
                    for lt in range(LT):
                        scalar.wait_ge(sems["mm1"], it * LT + lt + 1)
                        scalar.copy(v_sb[:, lt, 512:1024],
                                    ps[WAVE_B_BANKS[lt]][:, :]).then_inc(sems["cpB"])
                    for h in range(G2):
                        if h % 2 != 1:
                            continue
                        scalar.wait_ge(sems["mm2"], it * G2 + h + 1)
                        scalar.copy(at_sb[:, h, :],
                                    ps[PH2_BANKS[h % 4]][:, :]).then_inc(sems["cp2s"])
                    for m in [1, 3, 5, 7]:
                        scalar.wait_ge(sems["mm3"], it * G3 + m + 1)
                        if it > 0:
                            scalar.wait_ge(sems[f"dmo{m}"], it * 16)
                        scalar.copy(o_sb[:, m, :],
                                    ps[PH3_BANKS[m % 4]][:, :]).then_inc(
                            sems["cp3s"])


    return nc


# ---------------- host side ----------------

_GRAPH_CACHE: dict = {}


def get_graph(iters: int = 1, banded: bool = True) -> bass.Bass:
    key = (iters, banded)
    if key not in _GRAPH_CACHE:
        _GRAPH_CACHE[key] = build_graph(iters, banded)
    return _GRAPH_CACHE[key]


class Runner:
    """Compile-once executor for one Bass graph across the 8 cores.

    Mirrors bass2jax.run_bass_via_pjrt but keeps the jitted callable so
    repeated invocations don't re-trace/re-compile.
    """

    def __init__(self, nc: bass.Bass, n_cores: int = N_CORES):
        import jax
        from jax.sharding import Mesh, PartitionSpec
        from jax.experimental.shard_map import shard_map
        from concourse import bass2jax, mybir as _mb

        bass2jax.install_neuronx_cc_hook()
        self.n_cores = n_cores

        partition_name = (nc.partition_id_tensor.name
                          if nc.partition_id_tensor else None)
        in_names, out_names, out_avals, zero_shapes = [], [], [], []
        for alloc in nc.m.functions[0].allocations:
            if not isinstance(alloc, _mb.MemoryLocationSet):
                continue
            name = alloc.memorylocations[0].name
            if alloc.kind == "ExternalInput":
                if name != partition_name:
                    in_names.append(name)
            elif alloc.kind == "ExternalOutput":
                out_names.append(name)
                shape = tuple(alloc.tensor_shape)
                dtype = _mb.dt.np(alloc.dtype)
                out_avals.append(jax.core.ShapedArray(shape, dtype))
                zero_shapes.append((shape, dtype))
        self.in_names = list(in_names)
        self.out_names = out_names
        self.out_avals = out_avals
        self.zero_shapes = zero_shapes
        n_params = len(in_names)
        all_names = in_names + out_names
        if partition_name is not None:
            all_names = all_names + [partition_name]

        def _body(*args):
            operands = list(args)
            if partition_name is not None:
                operands.append(bass2jax.partition_id_tensor())
            outs = bass2jax._bass_exec_p.bind(
                *operands,
                out_avals=tuple(out_avals),
                in_names=tuple(all_names),
                out_names=tuple(out_names),
                lowering_input_output_aliases=(),
                sim_require_finite=True,
                sim_require_nnan=True,
                nc=nc,
            )
            return tuple(outs)

        devices = jax.devices()[:n_cores]
        mesh = Mesh(np.asarray(devices), ("core",))
        self._mesh = mesh
        n_outs = len(out_names)
        self._fn = jax.jit(
            shard_map(_body, mesh=mesh,
                      in_specs=(PartitionSpec("core"),) * (n_params + n_outs),
                      out_specs=(PartitionSpec("core"),) * n_outs,
                      check_rep=False),
            donate_argnums=tuple(range(n_params, n_params + n_outs)),
            keep_unused=True,
        )

    def stage(self, in_maps):
        """device_put the concatenated inputs once; returns device arrays."""
        import jax
        concat_in = [
            np.concatenate([np.asarray(m[name]) for m in in_maps], axis=0)
            for name in self.in_names
        ]
        return [jax.device_put(a) for a in concat_in]

    def make_zeros(self):
        if not hasattr(self, "_zeros_fn"):
            import jax
            import jax.numpy as jnp
            from jax.sharding import NamedSharding, PartitionSpec
            shardings = tuple(
                NamedSharding(self._mesh, PartitionSpec("core"))
                for _ in self.zero_shapes)
            shapes = [((self.n_cores * s[0], *s[1:]), d)
                      for s, d in self.zero_shapes]

            def _mk():
                return tuple(jnp.zeros(sh, dt) for sh, dt in shapes)

            self._zeros_fn = jax.jit(_mk, out_shardings=shardings)
        return list(self._zeros_fn())

    def run_staged(self, dev_in, dev_zeros):
        return self._fn(*dev_in, *dev_zeros)

    def __call__(self, in_maps):
        out_arrs = self._fn(*self.stage(in_maps), *self.make_zeros())
        return [
            {name: np.asarray(out_arrs[i]).reshape(
                self.n_cores, *self.out_avals[i].shape)[c]
             for i, name in enumerate(self.out_names)}
            for c in range(self.n_cores)
        ]


_RUNNER_CACHE: dict = {}


def get_runner(iters: int = 1) -> "Runner":
    if iters not in _RUNNER_CACHE:
        _RUNNER_CACHE[iters] = Runner(get_graph(iters))
    return _RUNNER_CACHE[iters]


def make_in_maps(values: np.ndarray, input_weights: np.ndarray,
                 output_weight: np.ndarray) -> list:
    bf = ml_dtypes.bfloat16
    w1t = np.ascontiguousarray(input_weights.T).astype(bf)
    w2t = np.ascontiguousarray(output_weight.T).astype(bf)
    tpt = gauss_toeplitz_table()
    in_maps = []
    for core in range(N_CORES):
        b, c = divmod(core, 4)
        lo, hi = c * CHUNK - HALO_L, c * CHUNK + CHUNK + HALO_R
        src_lo, src_hi = max(lo, 0), min(hi, L)
        xt_pad = np.zeros((D, LPAD), dtype=bf)
        xt_pad[:, src_lo - lo:src_hi - lo] = values[b, src_lo:src_hi, :].T.astype(bf)
        in_maps.append({"xt": xt_pad, "w1t": w1t, "w2t": w2t, "tp": tpt})
    return in_maps


def assemble(results: list) -> np.ndarray:
    out = np.empty((B, L, D), dtype=np.float32)
    for core in range(N_CORES):
        b, c = divmod(core, 4)
        out[b, c * CHUNK:(c + 1) * CHUNK, :] = \
            results[core]["out"].T.astype(np.float32)
    return out


def kernel(values: np.ndarray, input_weights: np.ndarray,
           output_weight: np.ndarray) -> np.ndarray:
    in_maps = make_in_maps(values, input_weights, output_weight)
    try:
        return assemble(get_runner(1)(in_maps))
    except Exception:
        # fallback: canonical SPMD path (re-traces per call but always works)
        res = run_bass_kernel_spmd(get_graph(1), in_maps,
                                   core_ids=list(range(N_CORES)))
        return assemble(res.results)



# revision 33
# speedup vs baseline: 1.0931x; 1.0931x over previous
"""Trainium2 Bass kernel for nn_Attention (Gaussian banded attention).

Math (reference):
    v = values @ input_weights.T                      # [B,L,D]
    probs[h,q,k] = N(k - q - off_h; std_h)            # Gaussian, depends on k-q only
    attended[b,h,q,:] = sum_k probs[h,q,k] v[b,k,h*pd:(h+1)*pd]
    out = attended_merged @ output_weight.T           # [B,L,D]

Key structural facts exploited:
  - probs is a banded Toeplitz matrix per head (6-sigma truncation), so
    attention is a narrow depthwise convolution along L; no [L,L] matmul.
  - Batch x L sharding is embarrassingly parallel given a halo of
    56 backward / 40 forward input rows (row-wise projection, no bias).
  - The two dense [1024,1024] projections run as fp8(e4m3) DoubleRow
    matmuls (0.5 cycles/row, 256-deep contraction = 4x bf16 FLOP rate),
    error-compensated with a 3-term split:
        x @ w ~= Xh@Wh + Xl@Wh + Xh@Wl,
    where Xh = fp8(x*s), Xl = fp8(x*s - Xh) (likewise W). Power-of-2
    scales keep everything in fp8's normal range and are folded into the
    host-prepared tables / final host descale, so no on-device rescaling
    is needed. Measured end-to-end error ~3e-3 (better than 2e-2 gate).
  - The banded attention itself stays bf16 (windowed Toeplitz matmuls;
    DoubleRow would widen the windows and lose the benefit).

Sharding: 8 cores = (B=2) x (4 chunks of 512 rows of L). No collectives.
"""

import math
from contextlib import ExitStack

import numpy as np
import ml_dtypes

import concourse.bass as bass
from concourse import mybir
from concourse.bass_utils import run_bass_kernel_spmd

# ---- NEFF disk cache (keyed by BIR hash) to avoid recompiling identical
# graphs in fresh processes ----
import hashlib
import os
import shutil

_NEFF_CACHE_DIR = os.environ.get("NEFF_CACHE_DIR", "/root/neff_cache")


def _install_neff_cache():
    import concourse.bass_utils as _bu
    import concourse.bass2jax as _b2j
    if getattr(_bu, "_neff_cache_installed", False):
        return
    orig = _bu.compile_bir_kernel

    def cached(bir_json, tmpdir, neff_name="file.neff"):
        cpath = None
        try:
            os.makedirs(_NEFF_CACHE_DIR, exist_ok=True)
            key = hashlib.sha256(bir_json).hexdigest()[:32]
            cpath = os.path.join(_NEFF_CACHE_DIR, f"{key}.neff")
            dst = os.path.join(tmpdir, neff_name)
            if os.path.exists(cpath):
                shutil.copy(cpath, dst)
                return dst
        except OSError:
            cpath = None  # cache unusable; plain compile below
        path = orig(bir_json, tmpdir, neff_name)
        if cpath is not None:
            try:
                shutil.copy(path, cpath)
            except OSError:
                pass
        return path

    _bu.compile_bir_kernel = cached
    _b2j.compile_bir_kernel = cached
    _bu._neff_cache_installed = True


_install_neff_cache()

# ---------------- problem constants (hardcoded per spec) ----------------
B, L, D = 2, 2048, 1024
H, PD = 8, 128
ATTN_STD = np.array([1.0, 2.0, 4.0, 8.0, 1.0, 2.0, 4.0, 8.0], dtype=np.float64)
ATTN_OFFSET = np.array([-1.0, -2.0, -4.0, -8.0, -1.0, -2.0, -4.0, -8.0], dtype=np.float64)

N_CORES = 8
CHUNK = 512            # output rows per core
HALO_L, HALO_R = 56, 40
LPAD = 640             # 56 + 512 + 40 = 608, padded to 5*128
LT = 5                 # l-tiles of v (640 / 128)
KT = 8                 # d tiles (1024 / 128)
KP = 4                 # DoubleRow contraction pairs (1024 / 256)
NQ = CHUNK             # query columns per core

F8 = mybir.dt.float8e4
BF16 = mybir.dt.bfloat16
F32 = mybir.dt.float32
DR = mybir.MatmulPerfMode.DoubleRow

# power-of-2 scales for fp8 quantization; folded into host tables.
SX = 32.0              # x (values^T)
SW1 = 256.0            # input_weights^T
SW2 = 256.0            # output_weight^T
SA = 32.0              # attended (via tp table scale)
TP_SCALE = SA / (SX * SW1)     # folded into the Gaussian table
OUT_DESCALE = SA * SW2         # host divides the bf16 output by this

G2 = H                 # attention heads -> attendedT
G3 = KT                # proj2 d_out tiles -> outT
TP0, TPW = 408, 240    # tp column window (banded)


def gauss_toeplitz_packed() -> np.ndarray:
    """tp[p, h, c] = g_h(p - (TP0+c - 512) - 56) * TP_SCALE, [128, H, TPW] bf16.

    For v-tile t (rows k' = 128t + r of padded-local v) the attention rhs is
    tp[:, h, 512-128t+j0-TP0 : ...] so that rhs[r, q'] = g_h(128t + r - q' - 56)
    = probs[h, q, k].T in padded-local coordinates (scaled).
    """
    r = np.arange(128, dtype=np.float64)[:, None]
    m = TP0 + np.arange(TPW, dtype=np.float64)[None, :]
    delta = r - (m - 512.0) - 56.0  # = k - q
    tables = []
    for h in range(H):
        std, off = ATTN_STD[h], ATTN_OFFSET[h]
        z = (delta - off) / std
        g = np.exp(-0.5 * z * z) / (std * math.sqrt(2.0 * math.pi))
        g[np.abs(z) > 6.0] = 0.0
        tables.append(g * TP_SCALE)
    return np.stack(tables, axis=1).astype(ml_dtypes.bfloat16)  # [128, H, TPW]


def attn_windows(h: int):
    """Static (t, j0, j1) list: nonzero q-column window of v-tile t for head h,
    8-aligned. Coverage of [0,512) is guaranteed (window width > 128)."""
    std, off = int(ATTN_STD[h]), int(ATTN_OFFSET[h])
    wlo = -56 - off - 6 * std
    whi = 71 - off + 6 * std
    res = []
    for t in range(LT):
        j0 = max(0, 128 * t + wlo)
        j1 = min(NQ, 128 * t + whi + 1)
        if j0 >= j1:
            continue
        j0 = (j0 // 8) * 8
        j1 = min(NQ, ((j1 + 7) // 8) * 8)
        res.append((t, j0, j1))
    return res


class _Waiter:
    """Per-engine wait_ge deduplication: skip waits dominated by an earlier
    wait on the same semaphore (counts are monotone)."""

    def __init__(self, eng):
        self.eng = eng
        self.seen = {}

    def wait(self, sem, count):
        if self.seen.get(id(sem), -1) >= count:
            return
        self.seen[id(sem)] = count
        self.eng.wait_ge(sem, count)


def build_graph(iters: int = 1, banded: bool = True) -> bass.Bass:
    """One SPMD core program. iters>1 repeats the kernel (x/w1 DMAs
    re-issued; tp/w2 loaded once) with increasing semaphore thresholds.

    PE program order per iteration (single stream, no warmup needed --
    the cost-model p-state ramp expires by the time data arrives):
      wave A : v[:, 0:512] = x @ W1a, pair-outer, banks 0-4, fp8 DR x3 terms
      wave B1: v[:, 512:1024] lt=0,1,2 pair-outer, banks 5,6,7
      ph2 h0,h1,h2 (banks 2,3,4)  -- interleaved: only need wave-A copies
      wave B2: lt=3,4 pair-inner, banks 0,1
      ph2 h3 (bank 5), ph2 h4-7 (banks 2-5)
      ph3 m0-7 (banks 0,1,6,7), terms (w2h,ath),(w2l,ath),(w2h,atl);
            m=7 split into q-column subtiles for a short tail.
    Copies: DVE: waveA lt0-2, at hi/lo odd heads, out even m.
            Act: waveA lt3-4, waveB all, at_hi even heads, out odd m.
            Pool: at_lo even heads, out DMAs.
    """
    nc = bass.Bass()

    xhl = nc.declare_dram_parameter("xhl", [128, KP, 2, 2, LPAD], F8, isOutput=False)
    w1 = nc.declare_dram_parameter("w1", [128, 2, KP, 2, 2, 512], F8, isOutput=False)
    w2 = nc.declare_dram_parameter("w2", [128, 2, KP, 2, KT, 128], F8, isOutput=False)
    tp = nc.declare_dram_parameter("tp", [128, H, TPW], BF16, isOutput=False)
    nid = nc.declare_dram_parameter("nid", [128, 2, 2, 128], F8, isOutput=False)
    out = nc.declare_dram_parameter("out", [D, NQ], BF16, isOutput=True)

    M7SUB = [(0, 256), (256, 512)]     # q-column subtiles of the last group
    NMM3 = G3 + 1                      # mm3 increments per iteration

    with ExitStack() as ctx:
        e = ctx.enter_context
        xhl_sb = e(nc.sbuf_tensor("xhl_sb", [128, 2, KP, 2, 2, LPAD], F8))
        w1_sb = e(nc.sbuf_tensor("w1_sb", [128, 2, 2, KP, 2, 2, 512], F8))
        w2_sb = e(nc.sbuf_tensor("w2_sb", [128, 2, KP, 2, KT, 128], F8))
        tp_sb = e(nc.sbuf_tensor("tp_sb", [128, H, TPW], BF16))
        nid_sb = e(nc.sbuf_tensor("nid_sb", [128, 2, 2, 128], F8))
        v_sb = e(nc.sbuf_tensor("v_sb", [128, LT, D], BF16))
        ath_sb = e(nc.sbuf_tensor("ath_sb", [128, H, NQ], F8))
        atl_sb = e(nc.sbuf_tensor("atl_sb", [128, H, NQ], F8))
        o_sb = e(nc.sbuf_tensor("o_sb", [128, KT, NQ], BF16))
        ps = [e(nc.psum_tensor(f"ps{i}", [128, 512], F32)) for i in range(8)]

        sem_names = (["mmA", "mm1", "mm2", "mm3", "tp_a", "tp_b",
                      "cpA_v", "cpA_s", "cpB",
                      "cp2h_a", "cp2h_v", "cp2l_a", "cp2l_v",
                      "mm2b", "nid_d", "ath0",
                      "cp3v", "cp3s", "cp3t", "cpB4_v",
                      "w2h1", "w2h2", "w2l1", "w2l2"]
                     + [f"x_d{j}b{p}" for j in range(KP) for p in (0, 1)]
                     + [f"wa_d{j}b{p}" for j in range(KP) for p in (0, 1)]
                     + [f"wb_d{j}b{p}" for j in range(KP) for p in (0, 1)]
                     + [f"x_l0b{p}" for p in (0, 1)]
                     + [f"wa_l0b{p}" for p in (0, 1)]
                     + [f"dmo{m}" for m in range(G3)]
                     + ["dmo7b", "cp3u"])
        sems = {n: e(nc.semaphore(n)) for n in sem_names}

        B1_BANKS = [5, 6, 7]           # wave B lt=0,1,2
        B2_BANKS = [0, 1]              # wave B lt=3,4
        PH2_BANKS = [2, 3, 4, 5]
        PH3_BANKS = [0, 1, 6, 7]
        TERMS1 = ((0, 0), (1, 0), (0, 1))   # (t=x hilo, u=w hilo)

        def vtile_wait(w, it, t, nhalf):
            """wait for v_sb tile t, n-half (0: wave A cols, 1: wave B)."""
            if nhalf == 0:
                if t < 3:
                    w.wait(sems["cpA_v"], it * 3 + t + 1)
                else:
                    w.wait(sems["cpA_s"], it * 2 + t - 2)
            elif t < 4:
                w.wait(sems["cpB"], it * 4 + t + 1)
            else:
                w.wait(sems["cpB4_v"], it + 1)

        def ph2_head(tensor, w, it, h, parts=(0, LT)):
            bank = ps[PH2_BANKS[h % 4]]
            # bank WAR: banks 2,3,4 <- wave A lt2,3,4 copies; bank 5 <- wave
            # B lt0 copy; h>=4 <- at_lo of head h-4
            if h == 0:
                w.wait(sems["cpA_v"], it * 3 + 3)
            elif h == 1:
                w.wait(sems["cpA_s"], it * 2 + 1)
            elif h == 2:
                w.wait(sems["cpA_s"], it * 2 + 2)
            elif h == 3:
                w.wait(sems["cpB"], it * 4 + 1)
            elif h == 4:
                w.wait(sems["cp2l_a"], it * 4 + 1)   # lo0
            elif h == 5:
                w.wait(sems["cp2l_v"], it * 4 + 1)   # lo1
            elif h == 6:
                w.wait(sems["cp2l_a"], it * 4 + 2)   # lo2
            else:
                w.wait(sems["cp2l_v"], it * 4 + 2)   # lo3
            if it == 0:
                w.wait(sems["tp_a" if h < 4 else "tp_b"], 16)
            ph2_windows(tensor, w, it, h, bank, *parts)

        def ph2_windows(tensor, w, it, h, bank, part0, part1):
            """Emit windows wi in [part0, part1); start on wi==0, stop+mm2
            on the last window overall."""
            windows = attn_windows(h) if banded else [
                (t, 0, NQ) for t in range(LT)]
            for wi in range(part0, min(part1, len(windows))):
                t, j0, j1 = windows[wi]
                vtile_wait(w, it, t, h // 4)
                c0 = 512 - 128 * t + j0 - TP0
                c1 = 512 - 128 * t + j1 - TP0
                mm = tensor.matmul(
                    bank[:, j0:j1],
                    v_sb[:, t, 128 * h:128 * h + 128],
                    tp_sb[:, h, c0:c1],
                    start=(wi == 0), stop=(wi == len(windows) - 1),
                )
                if wi == len(windows) - 1:
                    mm.then_inc(sems["mm2"])

        def neg_ident(tensor, w, it, h):
            """psum[bank(h)] += (-I) @ ath[h]  ->  bank holds at_lo."""
            if h % 2 == 0:
                w.wait(sems["cp2h_a"], it * 4 + h // 2 + 1)
            else:
                w.wait(sems["cp2h_v"], it * 4 + (h - 1) // 2 + 1)
            if it == 0 and h == 0:
                w.wait(sems["nid_d"], 16)
            tensor.matmul(
                ps[PH2_BANKS[h % 4]][:, :],
                nid_sb[:, 0, :, :],
                ath_sb[:, h:h + 1, :].to_broadcast([128, 2, NQ]),
                start=False, stop=True,
                perf_mode=DR,
                skip_group_check=True,
            ).then_inc(sems["mm2b"])

        with nc.Block() as block:

            @block.sync
            def _(sync: bass.BassEngine):

                for it in range(iters):
                    buf = it % 2
                    if it > 1:
                        # xhl/w1 buffer reuse: wave B (last reader) of it-2
                        sync.wait_ge(sems["mm1"], (it - 1) * LT)
                    # pair 0 split fine for a fast first matmul
                    sync.dma_start(out=xhl_sb[:, buf, 0, 0], in_=xhl[:, 0, 0]
                                   ).then_inc(sems[f"x_d0b{buf}"], 16)
                    sync.dma_start(out=w1_sb[:, buf, 0, 0, 0], in_=w1[:, 0, 0, 0]
                                   ).then_inc(sems[f"wa_d0b{buf}"], 16)
                    sync.dma_start(out=xhl_sb[:, buf, 0, 1], in_=xhl[:, 0, 1]
                                   ).then_inc(sems[f"x_l0b{buf}"], 16)
                    sync.dma_start(out=w1_sb[:, buf, 0, 0, 1], in_=w1[:, 0, 0, 1]
                                   ).then_inc(sems[f"wa_l0b{buf}"], 16)
                    for j in range(1, KP):
                        sync.dma_start(out=xhl_sb[:, buf, j], in_=xhl[:, j]
                                       ).then_inc(sems[f"x_d{j}b{buf}"], 16)
                        sync.dma_start(out=w1_sb[:, buf, 0, j], in_=w1[:, 0, j]
                                       ).then_inc(sems[f"wa_d{j}b{buf}"], 16)
                    for j in range(KP):
                        if it == 0 and j == 3:
                            sync.dma_start(out=tp_sb[:, 0:4], in_=tp[:, 0:4]
                                           ).then_inc(sems["tp_a"], 16)
                        sync.dma_start(out=w1_sb[:, buf, 1, j], in_=w1[:, 1, j]
                                       ).then_inc(sems[f"wb_d{j}b{buf}"], 16)
                    if it == 0:
                        sync.dma_start(out=nid_sb[:], in_=nid[:]
                                       ).then_inc(sems["nid_d"], 16)
                        sync.dma_start(out=tp_sb[:, 4:8], in_=tp[:, 4:8]
                                       ).then_inc(sems["tp_b"], 16)
                        sync.dma_start(out=w2_sb[:, 0, 0:2], in_=w2[:, 0, 0:2]
                                       ).then_inc(sems["w2h1"], 16)
                        sync.dma_start(out=w2_sb[:, 0, 2:4], in_=w2[:, 0, 2:4]
                                       ).then_inc(sems["w2h2"], 16)
                        sync.dma_start(out=w2_sb[:, 1, 0:2], in_=w2[:, 1, 0:2]
                                       ).then_inc(sems["w2l1"], 16)
                        sync.dma_start(out=w2_sb[:, 1, 2:4], in_=w2[:, 1, 2:4]
                                       ).then_inc(sems["w2l2"], 16)

            @block.tensor
            def _(tensor: bass.BassEngine):
                w = _Waiter(tensor)
                for it in range(iters):
                    buf = it % 2
                    nth = (it // 2 + 1) * 16  # per-parity DMA count

                    # ---- wave A: pair-outer, banks 0-4 ----
                    for j in range(KP):
                        for ti, (t, u) in enumerate(TERMS1):
                            if j == 0:
                                if ti == 0:
                                    w.wait(sems[f"x_d0b{buf}"], nth)
                                    w.wait(sems[f"wa_d0b{buf}"], nth)
                                elif ti == 1:
                                    w.wait(sems[f"x_l0b{buf}"], nth)
                                else:
                                    w.wait(sems[f"wa_l0b{buf}"], nth)
                            else:
                                w.wait(sems[f"x_d{j}b{buf}"], nth)
                                w.wait(sems[f"wa_d{j}b{buf}"], nth)
                            for lt in range(LT):
                                if j == 0 and ti == 0 and it > 0:
                                    # banks 0,1 <- ph3 m4,m5 copies (prev it);
                                    # banks 2,3,4 <- at_lo h4,h5,h6 (prev it)
                                    if lt == 0:
                                        w.wait(sems["cp3v"], (it - 1) * 4 + 3)
                                    elif lt == 1:
                                        w.wait(sems["cp3s"], (it - 1) * 3 + 3)
                                    elif lt == 2:
                                        w.wait(sems["cp3u"], it)
                                    elif lt == 3:
                                        w.wait(sems["cp2l_v"], (it - 1) * 4 + 3)
                                    else:
                                        w.wait(sems["cp2l_a"], (it - 1) * 4 + 4)
                                mm = tensor.matmul(
                                    ps[lt][:, :],
                                    xhl_sb[:, buf, j, t, :,
                                           128 * lt:128 * lt + 128],
                                    w1_sb[:, buf, 0, j, u, :, :],
                                    start=(j == 0 and ti == 0),
                                    stop=(j == KP - 1 and ti == 2),
                                    perf_mode=DR,
                                )
                                if j == KP - 1 and ti == 2:
                                    mm.then_inc(sems["mmA"])

                    # ---- wave B1: lt=0,1,2 pair-outer, banks 5,6,7 ----
                    for j in range(KP):
                        w.wait(sems[f"wb_d{j}b{buf}"], nth)
                        for ti, (t, u) in enumerate(TERMS1):
                            for li, lt in enumerate((0, 1, 2)):
                                if j == 0 and ti == 0 and it > 0:
                                    if lt == 0:   # bank5 <- at_lo h7 prev
                                        w.wait(sems["cp2l_v"], (it - 1) * 4 + 4)
                                    elif lt == 1:  # bank6 <- ph3 m6 prev
                                        w.wait(sems["cp3v"], (it - 1) * 4 + 4)
                                    else:          # bank7 <- ph3 m7a prev
                                        w.wait(sems["cp3t"], it)
                                mm = tensor.matmul(
                                    ps[B1_BANKS[li]][:, :],
                                    xhl_sb[:, buf, j, t, :,
                                           128 * lt:128 * lt + 128],
                                    w1_sb[:, buf, 1, j, u, :, :],
                                    start=(j == 0 and ti == 0),
                                    stop=(j == KP - 1 and ti == 2),
                                    perf_mode=DR,
                                )
                                if j == KP - 1 and ti == 2:
                                    mm.then_inc(sems["mm1"])

                    # ---- ph2 h0-h3 (banks 2,3,4,5; h3's bank free after
                    # ---- the vB0 copy) ----
                    for h in (0, 1, 2, 3):
                        ph2_head(tensor, w, it, h)
                    neg_ident(tensor, w, it, 0)
                    neg_ident(tensor, w, it, 1)

                    # ---- wave B2: lt=3,4 pair-inner, banks 0,1 ----
                    for li, lt in enumerate((3, 4)):
                        if lt == 4:
                            neg_ident(tensor, w, it, 2)
                            neg_ident(tensor, w, it, 3)
                        bank = ps[B2_BANKS[li]]
                        w.wait(sems["cpA_v"], it * 3 + li + 1)
                        for j in range(KP):
                            for ti, (t, u) in enumerate(TERMS1):
                                mm = tensor.matmul(
                                    bank[:, :],
                                    xhl_sb[:, buf, j, t, :,
                                           128 * lt:128 * lt + 128],
                                    w1_sb[:, buf, 1, j, u, :, :],
                                    start=(j == 0 and ti == 0),
                                    stop=(j == KP - 1 and ti == 2),
                                    perf_mode=DR,
                                )
                                if j == KP - 1 and ti == 2:
                                    mm.then_inc(sems["mm1"])

                    # ---- ph2 h4-7; h4/h5 t=4 windows (which need the vB4
                    # ---- copy) deferred behind h5's earlier windows ----
                    ph2_head(tensor, w, it, 4, parts=(0, 4))
                    ph2_head(tensor, w, it, 5, parts=(0, 4))
                    ph2_head(tensor, w, it, 6, parts=(0, 3))
                    ph2_windows(tensor, w, it, 4, ps[PH2_BANKS[0]], 4, 5)
                    ph2_windows(tensor, w, it, 5, ps[PH2_BANKS[1]], 4, 5)
                    ph2_windows(tensor, w, it, 6, ps[PH2_BANKS[2]], 3, 5)
                    ph2_head(tensor, w, it, 7)

                    # ---- ph3: banks [0,1,6,7]; terms (w2h,ath),(w2l,ath),
                    # ---- (w2h,atl); m=7 split into q subtiles ----
                    TERMS3 = ((0, 0), (1, 0), (0, 1))   # (u, lo)
                    for m in range(G3):
                        bank = ps[PH3_BANKS[m % 4]]
                        if m == 0:
                            w.wait(sems["cpB"], it * 4 + 4)   # <- vB3
                        elif m == 1:
                            w.wait(sems["cpB4_v"], it + 1)    # <- vB4
                        elif m < 4:
                            w.wait(sems["cpB"], it * 4 + m)   # <- vB1, vB2
                        elif m % 2 == 0:
                            w.wait(sems["cp3v"], it * 4 + (m - 4) // 2 + 1)
                        else:
                            w.wait(sems["cp3s"], it * 3 + (m - 4) // 2 + 1)
                        subs = M7SUB if m == G3 - 1 else [(0, NQ)]
                        if m == 0:
                            SEQ3 = [(0, 0), (0, 1), (1, 0), (1, 1),
                                    ("ni", 4), ("ni", 5), (0, 2), (1, 2),
                                    ("ni", 6), ("ni", 7), (0, 3), (1, 3),
                                    (2, 0), (2, 1), (2, 2), (2, 3)]
                        else:
                            SEQ3 = ([(t, j) for j in (0, 1) for t in (0, 1, 2)]
                                    + [(0, 2), (1, 2), (0, 3), (1, 3),
                                       (2, 2), (2, 3)])
                        nmm = sum(1 for x in SEQ3 if x[0] != "ni")
                        for si, (a, b) in enumerate(subs):
                            if m == G3 - 1:
                                bank = ps[7] if si == 0 else ps[2]
                                if si == 1:
                                    # bank2 WAR: at_lo h4 copy (this iter)
                                    w.wait(sems["cp2l_a"], it * 4 + 3)
                            qq = 0
                            for (ti, j) in SEQ3:
                                if ti == "ni":
                                    if si == 0:
                                        neg_ident(tensor, w, it, j)
                                    continue
                                qi = qq
                                qq += 1
                                u, lo = TERMS3[ti]
                                rhs_t = atl_sb if lo else ath_sb
                                if m == 0 and si == 0:
                                    if ti == 0:
                                        w.wait(sems["cp2h_a"], it * 4 + j + 1)
                                        w.wait(sems["cp2h_v"], it * 4 + j + 1)
                                        if it == 0 and j == 0:
                                            w.wait(sems["w2h1"], 16)
                                        if it == 0 and j == 2:
                                            w.wait(sems["w2h2"], 16)
                                    elif ti == 1:
                                        if it == 0 and j == 0:
                                            w.wait(sems["w2l1"], 16)
                                        if it == 0 and j == 2:
                                            w.wait(sems["w2l2"], 16)
                                    else:
                                        w.wait(sems["cp2l_a"], it * 4 + j + 1)
                                        w.wait(sems["cp2l_v"], it * 4 + j + 1)
                                mm = tensor.matmul(
                                    bank[:, 0:b - a],
                                    w2_sb[:, u, j, :, m, :],
                                    rhs_t[:, 2 * j:2 * j + 2, a:b],
                                    start=(qi == 0),
                                    stop=(qi == nmm - 1),
                                    perf_mode=DR,
                                )
                                if qi == nmm - 1:
                                    mm.then_inc(sems["mm3"])

            @block.vector
            def _(vector: bass.BassEngine):
                w = _Waiter(vector)
                for it in range(iters):
                    # wave A copies lt0,1,2
                    for lt in range(3):
                        w.wait(sems["mmA"], it * LT + lt + 1)
                        vector.tensor_copy(
                            out=v_sb[:, lt, 0:512], in_=ps[lt][:, :],
                        ).then_inc(sems["cpA_v"])
                    # at hi+lo for odd heads; vB4 copy between h3 and h5
                    def at_hi(h):
                        w.wait(sems["mm2"], it * G2 + h + 1)
                        if it == 0 and h == 1:
                            w.wait(sems["ath0"], 1)
                        vector.tensor_copy(
                            out=ath_sb[:, h, :], in_=ps[PH2_BANKS[h % 4]][:, :],
                        ).then_inc(sems["cp2h_v"])

                    def at_lo(h):
                        w.wait(sems["mm2b"], it * G2 + h + 1)
                        vector.tensor_copy(
                            out=atl_sb[:, h, :],
                            in_=ps[PH2_BANKS[h % 4]][:, :],
                        ).then_inc(sems["cp2l_v"])

                    at_hi(1); at_lo(1)
                    at_hi(3); at_lo(3)
                    w.wait(sems["mm1"], it * LT + 5)
                    vector.tensor_copy(
                        out=v_sb[:, 4, 512:1024], in_=ps[1][:, :],
                    ).then_inc(sems["cpB4_v"])
                    at_hi(5); at_hi(7); at_lo(5); at_lo(7)
                    for mi, m in enumerate((0, 2, 4, 6)):
                        w.wait(sems["mm3"], it * NMM3 + m + 1)
                        if it > 0:
                            w.wait(sems[f"dmo{m}"], it * 16)
                        vector.tensor_copy(
                            out=o_sb[:, m, :],
                            in_=ps[PH3_BANKS[m % 4]][:, :],
                        ).then_inc(sems["cp3v"])
                    a, b = M7SUB[1]
                    w.wait(sems["mm3"], it * NMM3 + 9)
                    if it > 0:
                        w.wait(sems["dmo7b"], it * 16)
                    vector.tensor_copy(
                        out=o_sb[:, 7, a:b], in_=ps[2][:, 0:b - a],
                    ).then_inc(sems["cp3u"])


            @block.gpsimd
            def _(gpsimd: bass.BassEngine):
                w = _Waiter(gpsimd)
                gpsimd.memset(ath_sb[:], 0).then_inc(sems["ath0"])
                for it in range(iters):
                    for m in range(G3 - 1):
                        if m % 2 == 0:
                            w.wait(sems["cp3v"], it * 4 + m // 2 + 1)
                        else:
                            w.wait(sems["cp3s"], it * 3 + m // 2 + 1)
                        if it > 0:
                            w.wait(sems[f"dmo{m}"], it * 16)
                        gpsimd.dma_start(
                            out=out[128 * m:128 * m + 128, :],
                            in_=o_sb[:, m, :],
                        ).then_inc(sems[f"dmo{m}"], 16)
                    a, b = M7SUB[0]
                    w.wait(sems["cp3t"], it + 1)
                    if it > 0:
                        w.wait(sems["dmo7"], it * 16)
                    gpsimd.dma_start(
                        out=out[896:1024, a:b],
                        in_=o_sb[:, 7, a:b],
                    ).then_inc(sems["dmo7"], 16)
                    a, b = M7SUB[1]
                    w.wait(sems["cp3u"], it + 1)
                    if it > 0:
                        w.wait(sems["dmo7b"], it * 16)
                    gpsimd.dma_start(
                        out=out[896:1024, a:b],
                        in_=o_sb[:, 7, a:b],
                    ).then_inc(sems["dmo7b"], 16)
                for m in range(G3 - 1):
                    gpsimd.wait_ge(sems[f"dmo{m}"], iters * 16)
                gpsimd.wait_ge(sems["dmo7"], iters * 16)
                gpsimd.wait_ge(sems["dmo7b"], iters * 16)

            @block.scalar
            def _(scalar: bass.BassEngine):
                w = _Waiter(scalar)
                for it in range(iters):
                    # wave A copies lt3,4
                    for lt in (3, 4):
                        w.wait(sems["mmA"], it * LT + lt + 1)
                        scalar.copy(v_sb[:, lt, 0:512], ps[lt][:, :]
                                    ).then_inc(sems["cpA_s"])
                    # wave B copies + at_hi even, interleaved by readiness
                    BSEQ = [("vB", 0), ("hi", 0), ("vB", 1), ("hi", 2),
                            ("vB", 2), ("lo", 0), ("vB", 3), ("lo", 2),
                            ("hi", 4), ("hi", 6), ("lo", 4), ("lo", 6)]
                    for kind, i in BSEQ:
                        if kind == "vB":
                            w.wait(sems["mm1"], it * LT + i + 1)
                            bank = (ps[B1_BANKS[i]] if i < 3
                                    else ps[B2_BANKS[i - 3]])
                            scalar.copy(v_sb[:, i, 512:1024], bank[:, :]
                                        ).then_inc(sems["cpB"])
                        elif kind == "hi":
                            w.wait(sems["mm2"], it * G2 + i + 1)
                            if it == 0 and i == 0:
                                w.wait(sems["ath0"], 1)
                            scalar.copy(ath_sb[:, i, :],
                                        ps[PH2_BANKS[i % 4]][:, :]
                                        ).then_inc(sems["cp2h_a"])
                        else:
                            w.wait(sems["mm2b"], it * G2 + i + 1)
                            scalar.copy(atl_sb[:, i, :],
                                        ps[PH2_BANKS[i % 4]][:, :]
                                        ).then_inc(sems["cp2l_a"])
                    for m in (1, 3, 5):
                        w.wait(sems["mm3"], it * NMM3 + m + 1)
                        if it > 0:
                            w.wait(sems[f"dmo{m}"], it * 16)
                        scalar.copy(o_sb[:, m, :],
                                    ps[PH3_BANKS[m % 4]][:, :]
                                    ).then_inc(sems["cp3s"])
                    a, b = M7SUB[0]
                    w.wait(sems["mm3"], it * NMM3 + 8)
                    if it > 0:
                        w.wait(sems["dmo7"], it * 16)
                    scalar.copy(o_sb[:, 7, a:b], ps[7][:, 0:b - a]
                                ).then_inc(sems["cp3t"])

    return nc


# ---------------- host side ----------------

_GRAPH_CACHE: dict = {}


def get_graph(iters: int = 1, banded: bool = True) -> bass.Bass:
    key = (iters, banded)
    if key not in _GRAPH_CACHE:
        _GRAPH_CACHE[key] = build_graph(iters, banded)
    return _GRAPH_CACHE[key]


class Runner:
    """Compile-once executor for one Bass graph across the 8 cores.

    Mirrors bass2jax.run_bass_via_pjrt but keeps the jitted callable so
    repeated invocations don't re-trace/re-compile.
    """

    def __init__(self, nc: bass.Bass, n_cores: int = N_CORES):
        import jax
        from jax.sharding import Mesh, PartitionSpec
        from jax.experimental.shard_map import shard_map
        from concourse import bass2jax, mybir as _mb

        bass2jax.install_neuronx_cc_hook()
        self.n_cores = n_cores

        partition_name = (nc.partition_id_tensor.name
                          if nc.partition_id_tensor else None)
        in_names, out_names, out_avals, zero_shapes = [], [], [], []
        for alloc in nc.m.functions[0].allocations:
            if not isinstance(alloc, _mb.MemoryLocationSet):
                continue
            name = alloc.memorylocations[0].name
            if alloc.kind == "ExternalInput":
                if name != partition_name:
                    in_names.append(name)
            elif alloc.kind == "ExternalOutput":
                out_names.append(name)
                shape = tuple(alloc.tensor_shape)
                dtype = _mb.dt.np(alloc.dtype)
                out_avals.append(jax.core.ShapedArray(shape, dtype))
                zero_shapes.append((shape, dtype))
        self.in_names = list(in_names)
        self.out_names = out_names
        self.out_avals = out_avals
        self.zero_shapes = zero_shapes
        n_params = len(in_names)
        all_names = in_names + out_names
        if partition_name is not None:
            all_names = all_names + [partition_name]

        def _body(*args):
            operands = list(args)
            if partition_name is not None:
                operands.append(bass2jax.partition_id_tensor())
            outs = bass2jax._bass_exec_p.bind(
                *operands,
                out_avals=tuple(out_avals),
                in_names=tuple(all_names),
                out_names=tuple(out_names),
                lowering_input_output_aliases=(),
                sim_require_finite=True,
                sim_require_nnan=True,
                nc=nc,
            )
            return tuple(outs)

        devices = jax.devices()[:n_cores]
        mesh = Mesh(np.asarray(devices), ("core",))
        self._mesh = mesh
        n_outs = len(out_names)
        self._fn = jax.jit(
            shard_map(_body, mesh=mesh,
                      in_specs=(PartitionSpec("core"),) * (n_params + n_outs),
                      out_specs=(PartitionSpec("core"),) * n_outs,
                      check_rep=False),
            donate_argnums=tuple(range(n_params, n_params + n_outs)),
            keep_unused=True,
        )

    def stage(self, in_maps):
        """device_put the concatenated inputs once; returns device arrays."""
        import jax
        concat_in = [
            np.concatenate([np.asarray(m[name]) for m in in_maps], axis=0)
            for name in self.in_names
        ]
        return [jax.device_put(a) for a in concat_in]

    def make_zeros(self):
        if not hasattr(self, "_zeros_fn"):
            import jax
            import jax.numpy as jnp
            from jax.sharding import NamedSharding, PartitionSpec
            shardings = tuple(
                NamedSharding(self._mesh, PartitionSpec("core"))
                for _ in self.zero_shapes)
            shapes = [((self.n_cores * s[0], *s[1:]), d)
                      for s, d in self.zero_shapes]

            def _mk():
                return tuple(jnp.zeros(sh, dt) for sh, dt in shapes)

            self._zeros_fn = jax.jit(_mk, out_shardings=shardings)
        return list(self._zeros_fn())

    def run_staged(self, dev_in, dev_zeros):
        return self._fn(*dev_in, *dev_zeros)

    def __call__(self, in_maps):
        out_arrs = self._fn(*self.stage(in_maps), *self.make_zeros())
        return [
            {name: np.asarray(out_arrs[i]).reshape(
                self.n_cores, *self.out_avals[i].shape)[c]
             for i, name in enumerate(self.out_names)}
            for c in range(self.n_cores)
        ]


_RUNNER_CACHE: dict = {}


def get_runner(iters: int = 1) -> "Runner":
    if iters not in _RUNNER_CACHE:
        _RUNNER_CACHE[iters] = Runner(get_graph(iters))
    return _RUNNER_CACHE[iters]


def _fp8_split(a32: np.ndarray):
    """(hi, lo) fp8 e4m3 pair with hi + lo ~= a32 (already scaled)."""
    f8 = ml_dtypes.float8_e4m3
    hi = a32.astype(f8)
    lo = (a32 - hi.astype(np.float32)).astype(f8)
    return hi, lo


def _pack_pairs(a: np.ndarray) -> np.ndarray:
    """[1024, F] -> [128, KP, 2, F]: d -> (pair j, i, partition p)."""
    F = a.shape[1]
    return np.ascontiguousarray(
        a.reshape(KP, 2, 128, F).transpose(2, 0, 1, 3))


def make_in_maps(values: np.ndarray, input_weights: np.ndarray,
                 output_weight: np.ndarray) -> list:
    f8 = ml_dtypes.float8_e4m3
    w1h, w1l = _fp8_split(np.ascontiguousarray(input_weights.T) * SW1)
    w2h, w2l = _fp8_split(np.ascontiguousarray(output_weight.T) * SW2)
    # w1 layout [128, half, pair, hilo, i, n]
    w1_pack = np.empty((128, 2, KP, 2, 2, 512), dtype=f8)
    for half in range(2):
        w1_pack[:, half, :, 0] = _pack_pairs(w1h[:, 512 * half:512 * half + 512])
        w1_pack[:, half, :, 1] = _pack_pairs(w1l[:, 512 * half:512 * half + 512])
    w2_pack = np.empty((128, 2, KP, 2, KT, 128), dtype=f8)
    w2_pack[:, 0] = _pack_pairs(w2h).reshape(128, KP, 2, KT, 128)
    w2_pack[:, 1] = _pack_pairs(w2l).reshape(128, KP, 2, KT, 128)
    tpt = gauss_toeplitz_packed()
    nid = np.zeros((128, 2, 2, 128), dtype=f8)
    eye = -np.eye(128, dtype=np.float32)
    nid[:, 0, 0] = eye.astype(f8)
    nid[:, 1, 1] = eye.astype(f8)
    in_maps = []
    for core in range(N_CORES):
        b, c = divmod(core, 4)
        lo_r, hi_r = c * CHUNK - HALO_L, c * CHUNK + CHUNK + HALO_R
        src_lo, src_hi = max(lo_r, 0), min(hi_r, L)
        xt_pad = np.zeros((D, LPAD), dtype=np.float32)
        xt_pad[:, src_lo - lo_r:src_hi - lo_r] = \
            values[b, src_lo:src_hi, :].T * SX
        xh, xl = _fp8_split(xt_pad)
        x_pack = np.empty((128, KP, 2, 2, LPAD), dtype=f8)
        x_pack[:, :, 0] = _pack_pairs(xh)
        x_pack[:, :, 1] = _pack_pairs(xl)
        in_maps.append({"xhl": x_pack, "w1": w1_pack, "w2": w2_pack,
                        "tp": tpt, "nid": nid})
    return in_maps


def assemble(results: list) -> np.ndarray:
    out = np.empty((B, L, D), dtype=np.float32)
    for core in range(N_CORES):
        b, c = divmod(core, 4)
        out[b, c * CHUNK:(c + 1) * CHUNK, :] = \
            results[core]["out"].T.astype(np.float32) / OUT_DESCALE
    return out


def kernel(values: np.ndarray, input_weights: np.ndarray,
           output_weight: np.ndarray) -> np.ndarray:
    in_maps = make_in_maps(values, input_weights, output_weight)
    try:
        return assemble(get_runner(1)(in_maps))
    except Exception:
        # fallback: canonical SPMD path (re-traces per call but always works)
        res = run_bass_kernel_spmd(get_graph(1), in_maps,
                                   core_ids=list(range(N_CORES)))
        return assemble(res.results)


# revision 58
# speedup vs baseline: 1.2150x; 1.1115x over previous
"""Trainium2 Bass kernel for nn_Attention (Gaussian banded attention).

Math (reference):
    v = values @ input_weights.T                      # [B,L,D]
    probs[h,q,k] = N(k - q - off_h; std_h)            # Gaussian, depends on k-q only
    attended[b,h,q,:] = sum_k probs[h,q,k] v[b,k,h*pd:(h+1)*pd]
    out = attended_merged @ output_weight.T           # [B,L,D]

Key structural facts exploited:
  - probs is a banded Toeplitz matrix per head (6-sigma truncation), so
    attention is a narrow depthwise convolution along L; no [L,L] matmul.
  - Batch x L sharding is embarrassingly parallel given a halo of
    56 backward / 40 forward input rows (row-wise projection, no bias).
  - The two dense [1024,1024] projections run as fp8(e4m3) DoubleRow
    matmuls (0.5 cycles/row, 256-deep contraction = 4x bf16 FLOP rate),
    error-compensated with a 3-term split:
        x @ w ~= Xh@Wh + Xl@Wh + Xh@Wl,
    where Xh = fp8(x*s), Xl = fp8(x*s - Xh) (likewise W). Power-of-2
    scales keep everything in fp8's normal range and are folded into the
    host-prepared tables / final host descale, so no on-device rescaling
    is needed. Measured end-to-end error ~3e-3 (better than 2e-2 gate).
  - The banded attention itself stays bf16 (windowed Toeplitz matmuls;
    DoubleRow would widen the windows and lose the benefit).

Sharding: 8 cores = (B=2) x (4 chunks of 512 rows of L). No collectives.

Cost-model performance (CoreSim, TRN2 timing): 31,777 ns single execution
(vs 38,609 ns for the bf16 baseline); 26,411 ns/iter steady state
(vs 33,046). PE stream is gapless to within ~0.3us; single-shot additionally
pays ~2.4us of initial DMA latency and ~2.9us of output copy+DMA tail.
"""

import math
from contextlib import ExitStack

import numpy as np
import ml_dtypes

import concourse.bass as bass
from concourse import mybir
from concourse.bass_utils import run_bass_kernel_spmd

# ---- NEFF disk cache (keyed by BIR hash) to avoid recompiling identical
# graphs in fresh processes ----
import hashlib
import os
import shutil

_NEFF_CACHE_DIR = os.environ.get("NEFF_CACHE_DIR", "/root/neff_cache")


def _install_neff_cache():
    import concourse.bass_utils as _bu
    import concourse.bass2jax as _b2j
    if getattr(_bu, "_neff_cache_installed", False):
        return
    orig = _bu.compile_bir_kernel

    def cached(bir_json, tmpdir, neff_name="file.neff"):
        cpath = None
        try:
            os.makedirs(_NEFF_CACHE_DIR, exist_ok=True)
            key = hashlib.sha256(bir_json).hexdigest()[:32]
            cpath = os.path.join(_NEFF_CACHE_DIR, f"{key}.neff")
            dst = os.path.join(tmpdir, neff_name)
            if os.path.exists(cpath):
                shutil.copy(cpath, dst)
                return dst
        except OSError:
            cpath = None  # cache unusable; plain compile below
        path = orig(bir_json, tmpdir, neff_name)
        if cpath is not None:
            try:
                shutil.copy(path, cpath)
            except OSError:
                pass
        return path

    _bu.compile_bir_kernel = cached
    _b2j.compile_bir_kernel = cached
    _bu._neff_cache_installed = True


_install_neff_cache()

# ---------------- problem constants (hardcoded per spec) ----------------
B, L, D = 2, 2048, 1024
H, PD = 8, 128
ATTN_STD = np.array([1.0, 2.0, 4.0, 8.0, 1.0, 2.0, 4.0, 8.0], dtype=np.float64)
ATTN_OFFSET = np.array([-1.0, -2.0, -4.0, -8.0, -1.0, -2.0, -4.0, -8.0], dtype=np.float64)

N_CORES = 8
CHUNK = 512            # output rows per core
HALO_L, HALO_R = 56, 40
LPAD = 640             # 56 + 512 + 40 = 608, padded to 5*128
LT = 5                 # l-tiles of v (640 / 128)
KT = 8                 # d tiles (1024 / 128)
KP = 4                 # DoubleRow contraction pairs (1024 / 256)
NQ = CHUNK             # query columns per core

F8 = mybir.dt.float8e4
BF16 = mybir.dt.bfloat16
F32 = mybir.dt.float32
DR = mybir.MatmulPerfMode.DoubleRow

# power-of-2 scales for fp8 quantization; folded into host tables.
SX = 32.0              # x (values^T)
SW1 = 256.0            # input_weights^T
SW2 = 256.0            # output_weight^T
SA = 32.0              # attended (via tp table scale)
TP_SCALE = SA / (SX * SW1)     # folded into the Gaussian table
OUT_DESCALE = SA * SW2         # host divides the bf16 output by this

G2 = H                 # attention heads -> attendedT
G3 = KT                # proj2 d_out tiles -> outT
TP0, TPW = 408, 240    # tp column window (banded)


def gauss_toeplitz_packed() -> np.ndarray:
    """tp[p, h, c] = g_h(p - (TP0+c - 512) - 56) * TP_SCALE, [128, H, TPW] bf16.

    For v-tile t (rows k' = 128t + r of padded-local v) the attention rhs is
    tp[:, h, 512-128t+j0-TP0 : ...] so that rhs[r, q'] = g_h(128t + r - q' - 56)
    = probs[h, q, k].T in padded-local coordinates (scaled).
    """
    r = np.arange(128, dtype=np.float64)[:, None]
    m = TP0 + np.arange(TPW, dtype=np.float64)[None, :]
    delta = r - (m - 512.0) - 56.0  # = k - q
    tables = []
    for h in range(H):
        std, off = ATTN_STD[h], ATTN_OFFSET[h]
        z = (delta - off) / std
        g = np.exp(-0.5 * z * z) / (std * math.sqrt(2.0 * math.pi))
        g[np.abs(z) > 4.5] = 0.0
        tables.append(g * TP_SCALE)
    return np.stack(tables, axis=1).astype(ml_dtypes.bfloat16)  # [128, H, TPW]


def attn_windows(h: int):
    """Static (t, j0, j1) list: nonzero q-column window of v-tile t for head h,
    8-aligned. Coverage of [0,512) is guaranteed (window width > 128)."""
    std, off = int(ATTN_STD[h]), int(ATTN_OFFSET[h])
    wlo = -56 - off - int(4.5 * std)
    whi = 71 - off + int(4.5 * std)
    res = []
    for t in range(LT):
        j0 = max(0, 128 * t + wlo)
        j1 = min(NQ, 128 * t + whi + 1)
        if j0 >= j1:
            continue
        j0 = (j0 // 8) * 8
        j1 = min(NQ, ((j1 + 7) // 8) * 8)
        res.append((t, j0, j1))
    return res


class _Waiter:
    """Per-engine wait_ge deduplication: skip waits dominated by an earlier
    wait on the same semaphore (counts are monotone)."""

    def __init__(self, eng):
        self.eng = eng
        self.seen = {}

    def wait(self, sem, count):
        if self.seen.get(id(sem), -1) >= count:
            return
        self.seen[id(sem)] = count
        self.eng.wait_ge(sem, count)


def build_graph(iters: int = 1, banded: bool = True) -> bass.Bass:
    """One SPMD core program. iters>1 repeats the kernel (x/w1 DMAs
    re-issued; tp/w2 loaded once) with increasing semaphore thresholds.

    PE program order per iteration (single stream, no warmup needed --
    the cost-model p-state ramp expires by the time data arrives):
      wave A : v[:, 0:512] = x @ W1a, pair-outer, banks 0-4, fp8 DR x3 terms
      wave B1: v[:, 512:1024] lt=0,1,2 pair-outer, banks 5,6,7
      ph2 h0,h1,h2 (banks 2,3,4)  -- interleaved: only need wave-A copies
      wave B2: lt=3,4 pair-inner, banks 0,1
      ph2 h3 (bank 5), ph2 h4-7 (banks 2-5)
      ph3 m0-7 (banks 0,1,6,7), terms (w2h,ath),(w2l,ath),(w2h,atl);
            m=7 split into q-column subtiles for a short tail.
    Copies: DVE: waveA lt0-2, at hi/lo odd heads, out even m.
            Act: waveA lt3-4, waveB all, at_hi even heads, out odd m.
            Pool: at_lo even heads, out DMAs.
    """
    nc = bass.Bass()

    xhl = nc.declare_dram_parameter("xhl", [128, KP, 2, 2, LPAD], F8, isOutput=False)
    w1 = nc.declare_dram_parameter("w1", [128, 2, KP, 2, 2, 512], F8, isOutput=False)
    w2 = nc.declare_dram_parameter("w2", [128, 2, KP, 2, KT, 128], F8, isOutput=False)
    tp = nc.declare_dram_parameter("tp", [128, H, TPW], BF16, isOutput=False)
    nid = nc.declare_dram_parameter("nid", [128, 2, 2, 128], F8, isOutput=False)
    out = nc.declare_dram_parameter("out", [D, NQ], BF16, isOutput=True)

    M7SUB = [(0, 352), (352, 512)]     # q-column subtiles of the last group
    NMM3 = G3 + 1                      # mm3 increments per iteration

    with ExitStack() as ctx:
        e = ctx.enter_context
        xhl_sb = e(nc.sbuf_tensor("xhl_sb", [128, 2, KP, 2, 2, LPAD], F8))
        w1_sb = e(nc.sbuf_tensor("w1_sb", [128, 2, 2, KP, 2, 2, 512], F8))
        w2_sb = e(nc.sbuf_tensor("w2_sb", [128, 2, KP, 2, KT, 128], F8))
        tp_sb = e(nc.sbuf_tensor("tp_sb", [128, H, TPW], BF16))
        nid_sb = e(nc.sbuf_tensor("nid_sb", [128, 2, 2, 128], F8))
        v_sb = e(nc.sbuf_tensor("v_sb", [128, LT, D], BF16))
        ath_sb = e(nc.sbuf_tensor("ath_sb", [128, H, NQ], F8))
        atl_sb = e(nc.sbuf_tensor("atl_sb", [128, H, NQ], F8))
        o_sb = e(nc.sbuf_tensor("o_sb", [128, KT, NQ], BF16))
        ps = [e(nc.psum_tensor(f"ps{i}", [128, 512], F32)) for i in range(8)]

        sem_names = (["mmA", "mm1", "mm2", "mm3", "tp_a", "tp_b",
                      "cpA_v", "cpA_s", "cpB", "cpB_v",
                      "cp2h_a", "cp2h_v", "cp2l_a", "cp2l_v",
                      "mm2b", "nid_d", "ath0",
                      "cp3v", "cp3s", "cp3t", "cpB4_v",
                      "w2h1", "w2h2", "w2l1", "w2l2"]
                     + [f"x_d{j}b{p}" for j in range(KP) for p in (0, 1)]
                     + [f"wa_d{j}b{p}" for j in range(KP) for p in (0, 1)]
                     + [f"wb_d{j}b{p}" for j in range(KP) for p in (0, 1)]
                     + [f"x_l0b{p}" for p in (0, 1)]
                     + [f"wa_l0b{p}" for p in (0, 1)]
                     + [f"dmo{m}" for m in range(G3)]
                     + ["dmo7b", "cp3u"])
        sems = {n: e(nc.semaphore(n)) for n in sem_names}

        B1_BANKS = [5, 6, 7]           # wave B lt=0,1,2
        B2_BANKS = [0, 1]              # wave B lt=3,4
        PH2_BANKS = [2, 3, 4, 5]
        PH3_BANKS = [0, 1, 6, 7]
        TERMS1 = ((0, 0), (1, 0), (0, 1))   # (t=x hilo, u=w hilo)

        def vtile_wait(w, it, t, h):
            """wait for v_sb tile t for head h's column slice."""
            nhalf = h // 4
            if nhalf == 0:
                if t < 3:
                    w.wait(sems["cpA_v"], it * 3 + t + 1)
                else:
                    w.wait(sems["cpA_s"], it * 2 + t - 2)
            elif t < 2:
                w.wait(sems["cpB_v"], it * 2 + t + 1)
            elif t < 4:
                w.wait(sems["cpB"], it * 2 + t - 1)
            elif h < 6:
                w.wait(sems["cpB4_v"], it * 2 + 1)
            else:
                w.wait(sems["cpB4_v"], it * 2 + 2)

        def ph2_head(tensor, w, it, h, parts=(0, LT)):
            bank = ps[PH2_BANKS[h % 4]]
            # bank WAR: banks 2,3,4 <- wave A lt2,3,4 copies; bank 5 <- wave
            # B lt0 copy; h>=4 <- at_lo of head h-4
            if h == 0:
                w.wait(sems["cpA_v"], it * 3 + 3)
            elif h == 1:
                w.wait(sems["cpA_s"], it * 2 + 1)
            elif h == 2:
                w.wait(sems["cpA_s"], it * 2 + 2)
            elif h == 3:
                w.wait(sems["cpB_v"], it * 2 + 1)
            elif h == 4:
                w.wait(sems["cp2l_a"], it * 4 + 1)   # lo0
            elif h == 5:
                w.wait(sems["cp2l_v"], it * 4 + 1)   # lo1
            elif h == 6:
                w.wait(sems["cp2l_a"], it * 4 + 2)   # lo2
            else:
                w.wait(sems["cp2l_v"], it * 4 + 2)   # lo3
            if it == 0:
                w.wait(sems["tp_a" if h < 4 else "tp_b"], 16)
            ph2_windows(tensor, w, it, h, bank, *parts)

        def ph2_windows(tensor, w, it, h, bank, part0, part1):
            """Emit windows wi in [part0, part1); start on wi==0, stop+mm2
            on the last window overall."""
            windows = attn_windows(h) if banded else [
                (t, 0, NQ) for t in range(LT)]
            for wi in range(part0, min(part1, len(windows))):
                t, j0, j1 = windows[wi]
                vtile_wait(w, it, t, h)
                c0 = 512 - 128 * t + j0 - TP0
                c1 = 512 - 128 * t + j1 - TP0
                mm = tensor.matmul(
                    bank[:, j0:j1],
                    v_sb[:, t, 128 * h:128 * h + 128],
                    tp_sb[:, h, c0:c1],
                    start=(wi == 0), stop=(wi == len(windows) - 1),
                )
                if wi == len(windows) - 1:
                    mm.then_inc(sems["mm2"])

        def neg_ident(tensor, w, it, h):
            """psum[bank(h)] += (-I) @ ath[h] -> bank holds at_lo.
            Even heads only (their at_lo copy runs on Act, which cannot
            subtract); odd heads' DVE computes the residual directly."""
            assert h % 2 == 0
            w.wait(sems["cp2h_a"], it * 4 + h // 2 + 1)
            if it == 0 and h == 0:
                w.wait(sems["nid_d"], 16)
            tensor.matmul(
                ps[PH2_BANKS[h % 4]][:, :],
                nid_sb[:, 0, :, :],
                ath_sb[:, h:h + 1, :].to_broadcast([128, 2, NQ]),
                start=False, stop=True,
                perf_mode=DR,
                skip_group_check=True,
            ).then_inc(sems["mm2b"])

        with nc.Block() as block:

            @block.sync
            def _(sync: bass.BassEngine):

                for it in range(iters):
                    buf = it % 2
                    if it > 1:
                        # xhl/w1 buffer reuse: wave B (last reader) of it-2
                        sync.wait_ge(sems["mm1"], (it - 1) * LT)
                    # pair 0 split fine for a fast first matmul
                    sync.dma_start(out=xhl_sb[:, buf, 0, 0], in_=xhl[:, 0, 0]
                                   ).then_inc(sems[f"x_d0b{buf}"], 16)
                    sync.dma_start(out=w1_sb[:, buf, 0, 0, 0], in_=w1[:, 0, 0, 0]
                                   ).then_inc(sems[f"wa_d0b{buf}"], 16)
                    sync.dma_start(out=xhl_sb[:, buf, 0, 1], in_=xhl[:, 0, 1]
                                   ).then_inc(sems[f"x_l0b{buf}"], 16)
                    sync.dma_start(out=w1_sb[:, buf, 0, 0, 1], in_=w1[:, 0, 0, 1]
                                   ).then_inc(sems[f"wa_l0b{buf}"], 16)
                    for j in range(1, KP):
                        sync.dma_start(out=xhl_sb[:, buf, j], in_=xhl[:, j]
                                       ).then_inc(sems[f"x_d{j}b{buf}"], 16)
                        sync.dma_start(out=w1_sb[:, buf, 0, j], in_=w1[:, 0, j]
                                       ).then_inc(sems[f"wa_d{j}b{buf}"], 16)
                    for j in range(KP):
                        if it == 0 and j == 3:
                            sync.dma_start(out=tp_sb[:, 0:4], in_=tp[:, 0:4]
                                           ).then_inc(sems["tp_a"], 16)
                        sync.dma_start(out=w1_sb[:, buf, 1, j], in_=w1[:, 1, j]
                                       ).then_inc(sems[f"wb_d{j}b{buf}"], 16)
                    if it == 0:
                        sync.dma_start(out=nid_sb[:], in_=nid[:]
                                       ).then_inc(sems["nid_d"], 16)
                        sync.dma_start(out=tp_sb[:, 4:8], in_=tp[:, 4:8]
                                       ).then_inc(sems["tp_b"], 16)
                        sync.dma_start(out=w2_sb[:, 0, 0:2], in_=w2[:, 0, 0:2]
                                       ).then_inc(sems["w2h1"], 16)
                        sync.dma_start(out=w2_sb[:, 0, 2:4], in_=w2[:, 0, 2:4]
                                       ).then_inc(sems["w2h2"], 16)
                        sync.dma_start(out=w2_sb[:, 1, 0:2], in_=w2[:, 1, 0:2]
                                       ).then_inc(sems["w2l1"], 16)
                        sync.dma_start(out=w2_sb[:, 1, 2:4], in_=w2[:, 1, 2:4]
                                       ).then_inc(sems["w2l2"], 16)

            @block.tensor
            def _(tensor: bass.BassEngine):
                w = _Waiter(tensor)
                for it in range(iters):
                    buf = it % 2
                    nth = (it // 2 + 1) * 16  # per-parity DMA count

                    # ---- wave A: pair-outer, banks 0-4 ----
                    for j in range(KP):
                        for ti, (t, u) in enumerate(TERMS1):
                            if j == 0:
                                if ti == 0:
                                    w.wait(sems[f"x_d0b{buf}"], nth)
                                    w.wait(sems[f"wa_d0b{buf}"], nth)
                                elif ti == 1:
                                    w.wait(sems[f"x_l0b{buf}"], nth)
                                else:
                                    w.wait(sems[f"wa_l0b{buf}"], nth)
                            else:
                                w.wait(sems[f"x_d{j}b{buf}"], nth)
                                w.wait(sems[f"wa_d{j}b{buf}"], nth)
                            for lt in range(LT):
                                if j == 0 and ti == 0 and it > 0:
                                    # banks 0,1 <- ph3 m4,m5 copies (prev it);
                                    # banks 2,3,4 <- at_lo h4,h5,h6 (prev it)
                                    if lt == 0:
                                        w.wait(sems["cp3v"], (it - 1) * 4 + 3)
                                    elif lt == 1:
                                        w.wait(sems["cp3s"], (it - 1) * 3 + 3)
                                    elif lt == 2:
                                        w.wait(sems["cp3u"], it)
                                    elif lt == 3:
                                        w.wait(sems["cp2l_v"], (it - 1) * 4 + 3)
                                    else:
                                        w.wait(sems["cp2l_a"], (it - 1) * 4 + 4)
                                mm = tensor.matmul(
                                    ps[lt][:, :],
                                    xhl_sb[:, buf, j, t, :,
                                           128 * lt:128 * lt + 128],
                                    w1_sb[:, buf, 0, j, u, :, :],
                                    start=(j == 0 and ti == 0),
                                    stop=(j == KP - 1 and ti == 2),
                                    perf_mode=DR,
                                )
                                if j == KP - 1 and ti == 2:
                                    mm.then_inc(sems["mmA"])

                    # ---- wave B1: lt=0,1,2 pair-outer, banks 5,6,7 ----
                    for j in range(KP):
                        w.wait(sems[f"wb_d{j}b{buf}"], nth)
                        for ti, (t, u) in enumerate(TERMS1):
                            for li, lt in enumerate((0, 1, 2)):
                                if j == 0 and ti == 0 and it > 0:
                                    if lt == 0:   # bank5 <- at_lo h7 prev
                                        w.wait(sems["cp2l_v"], (it - 1) * 4 + 4)
                                    elif lt == 1:  # bank6 <- ph3 m6 prev
                                        w.wait(sems["cp3v"], (it - 1) * 4 + 4)
                                    else:          # bank7 <- ph3 m7a prev
                                        w.wait(sems["cp3t"], it)
                                mm = tensor.matmul(
                                    ps[B1_BANKS[li]][:, :],
                                    xhl_sb[:, buf, j, t, :,
                                           128 * lt:128 * lt + 128],
                                    w1_sb[:, buf, 1, j, u, :, :],
                                    start=(j == 0 and ti == 0),
                                    stop=(j == KP - 1 and ti == 2),
                                    perf_mode=DR,
                                )
                                if j == KP - 1 and ti == 2:
                                    mm.then_inc(sems["mm1"])

                    # ---- ph2 h0-h3 (banks 2,3,4,5; h3's bank free after
                    # ---- the vB0 copy) ----
                    for h in (0, 1, 2, 3):
                        ph2_head(tensor, w, it, h)
                    neg_ident(tensor, w, it, 0)

                    # ---- wave B2: lt=3,4 pair-inner, banks 0,1 ----
                    for li, lt in enumerate((3, 4)):
                        if lt == 4:
                            neg_ident(tensor, w, it, 2)
                        bank = ps[B2_BANKS[li]]
                        w.wait(sems["cpA_v"], it * 3 + li + 1)
                        for j in range(KP):
                            for ti, (t, u) in enumerate(TERMS1):
                                mm = tensor.matmul(
                                    bank[:, :],
                                    xhl_sb[:, buf, j, t, :,
                                           128 * lt:128 * lt + 128],
                                    w1_sb[:, buf, 1, j, u, :, :],
                                    start=(j == 0 and ti == 0),
                                    stop=(j == KP - 1 and ti == 2),
                                    perf_mode=DR,
                                )
                                if j == KP - 1 and ti == 2:
                                    mm.then_inc(sems["mm1"])

                    # ---- ph3: banks [0,1,6,7]; terms (w2h,ath),(w2l,ath),
                    # ---- (w2h,atl); m0+m1 interleaved around the late-head
                    # ---- at chains; m=7 split into q subtiles ----
                    TERMS3 = ((0, 0), (1, 0), (0, 1))   # (u, lo)

                    def ph3_mm(m, ti, j, bank, a, b, first, last):
                        u, lo = TERMS3[ti]
                        rhs_t = atl_sb if lo else ath_sb
                        if m <= 1:
                            if ti == 0:
                                w.wait(sems["cp2h_a"], it * 4 + j + 1)
                                w.wait(sems["cp2h_v"], it * 4 + j + 1)
                                if it == 0 and j == 0:
                                    w.wait(sems["w2h1"], 16)
                                if it == 0 and j == 2:
                                    w.wait(sems["w2h2"], 16)
                            elif ti == 1:
                                if it == 0 and j == 0:
                                    w.wait(sems["w2l1"], 16)
                                if it == 0 and j == 2:
                                    w.wait(sems["w2l2"], 16)
                            else:
                                w.wait(sems["cp2l_a"], it * 4 + j + 1)
                                w.wait(sems["cp2l_v"], it * 4 + j + 1)
                        mm = tensor.matmul(
                            bank[:, 0:b - a],
                            w2_sb[:, u, j, :, m, :],
                            rhs_t[:, 2 * j:2 * j + 2, a:b],
                            start=first, stop=last,
                            perf_mode=DR,
                        )
                        if last:
                            mm.then_inc(sems["mm3"])


                    # ---- ph2 h4-7; h4/h5 t=4 windows (which need the vB4
                    # ---- copy) deferred behind h5's earlier windows ----
                    ph2_head(tensor, w, it, 4, parts=(0, 4))
                    ph2_head(tensor, w, it, 5, parts=(0, 4))
                    ph2_head(tensor, w, it, 6, parts=(0, 3))
                    # fill the cpB4_v wait with m0's early matmuls
                    seen01 = {0: 0, 1: 0}
                    w.wait(sems["cpB"], it * 2 + 2)       # bank0 <- vB3
                    for (ti, j) in ((0, 0), (0, 1), (1, 0), (1, 1)):
                        seen01[0] += 1
                        ph3_mm(0, ti, j, ps[PH3_BANKS[0]], 0, NQ,
                               seen01[0] == 1, False)
                    ph2_windows(tensor, w, it, 4, ps[PH2_BANKS[0]], 4, 5)
                    ph2_windows(tensor, w, it, 5, ps[PH2_BANKS[1]], 4, 5)
                    ph2_windows(tensor, w, it, 6, ps[PH2_BANKS[2]], 3, 5)
                    ph2_head(tensor, w, it, 7)
                    neg_ident(tensor, w, it, 4)

                    # m0 (bank 0) + m1 (bank 1) interleaved; m0's first
                    # four matmuls already ran in the ph2 deferral window
                    SCHED01 = [(1, 0, 0), (1, 0, 1),
                               (1, 1, 0), (1, 1, 1),
                               ("ni", 6, 0),
                               (0, 2, 0), (0, 2, 1), (1, 2, 0), (1, 2, 1),
                               (0, 0, 2), (0, 1, 2), (1, 0, 2), (1, 1, 2),
                               (0, 0, 3), (0, 1, 3), (1, 0, 3), (1, 1, 3),
                               (0, 2, 2), (1, 2, 2), (0, 2, 3), (1, 2, 3)]
                    w.wait(sems["cpB4_v"], it * 2 + 2)    # bank1 <- vB4
                    total01 = {0: 12, 1: 12}
                    for (m, ti, j) in SCHED01:
                        if m == "ni":
                            neg_ident(tensor, w, it, ti)
                            continue
                        seen01[m] += 1
                        ph3_mm(m, ti, j, ps[PH3_BANKS[m]], 0, NQ,
                               seen01[m] == 1, seen01[m] == total01[m])

                    SEQ3 = ([(t, j) for j in (0, 1) for t in (0, 1, 2)]
                            + [(0, 2), (1, 2), (0, 3), (1, 3),
                               (2, 2), (2, 3)])
                    for m in range(2, G3):
                        bank = ps[PH3_BANKS[m % 4]]
                        if m == 2:
                            w.wait(sems["cpB_v"], it * 2 + 2)   # <- vB1
                        elif m == 3:
                            w.wait(sems["cpB"], it * 2 + 1)     # <- vB2
                        elif m % 2 == 0:
                            w.wait(sems["cp3v"], it * 4 + (m - 4) // 2 + 1)
                        else:
                            w.wait(sems["cp3s"], it * 3 + (m - 4) // 2 + 1)
                        subs = M7SUB if m == G3 - 1 else [(0, NQ)]
                        for si, (a, b) in enumerate(subs):
                            if m == G3 - 1:
                                bank = ps[7] if si == 0 else ps[2]
                                if si == 1:
                                    # bank2 WAR: at_lo h4 copy (this iter)
                                    w.wait(sems["cp2l_a"], it * 4 + 3)
                            for qi, (ti, j) in enumerate(SEQ3):
                                ph3_mm(m, ti, j, bank, a, b,
                                       qi == 0, qi == len(SEQ3) - 1)

            @block.vector
            def _(vector: bass.BassEngine):
                w = _Waiter(vector)
                for it in range(iters):
                    # wave A copies lt0,1,2
                    for lt in range(3):
                        w.wait(sems["mmA"], it * LT + lt + 1)
                        vector.tensor_copy(
                            out=v_sb[:, lt, 0:512], in_=ps[lt][:, :],
                        ).then_inc(sems["cpA_v"])
                    # wave B lt0, lt1 copies (DVE is idle here; Act is the
                    # saturated engine in this window)
                    for li in (0, 1):
                        w.wait(sems["mm1"], it * LT + li + 1)
                        vector.tensor_copy(
                            out=v_sb[:, li, 512:1024],
                            in_=ps[B1_BANKS[li]][:, :],
                        ).then_inc(sems["cpB_v"])

                    # at hi+lo for odd heads; vB4 copy between h3 and h5
                    def at_hi(h):
                        w.wait(sems["mm2"], it * G2 + h + 1)
                        if it == 0 and h == 1:
                            w.wait(sems["ath0"], 1)
                        vector.tensor_copy(
                            out=ath_sb[:, h, :], in_=ps[PH2_BANKS[h % 4]][:, :],
                        ).then_inc(sems["cp2h_v"])

                    def at_lo(h):
                        w.wait(sems["cp2h_v"], it * 4 + (h - 1) // 2 + 1)
                        vector.tensor_sub(
                            atl_sb[:, h, :], ps[PH2_BANKS[h % 4]][:, :],
                            ath_sb[:, h, :],
                        ).then_inc(sems["cp2l_v"])

                    at_hi(1); at_lo(1)
                    at_hi(3); at_lo(3)
                    w.wait(sems["mm1"], it * LT + 5)
                    vector.tensor_copy(
                        out=v_sb[:, 4, 512:768], in_=ps[1][:, 0:256],
                    ).then_inc(sems["cpB4_v"])
                    vector.tensor_copy(
                        out=v_sb[:, 4, 768:1024], in_=ps[1][:, 256:512],
                    ).then_inc(sems["cpB4_v"])
                    at_hi(5); at_hi(7); at_lo(5); at_lo(7)
                    for mi, m in enumerate((0, 2, 4, 6)):
                        w.wait(sems["mm3"], it * NMM3 + m + 1)
                        if it > 0:
                            w.wait(sems[f"dmo{m}"], it * 16)
                        vector.tensor_copy(
                            out=o_sb[:, m, :],
                            in_=ps[PH3_BANKS[m % 4]][:, :],
                        ).then_inc(sems["cp3v"])
                    a, b = M7SUB[0]
                    w.wait(sems["mm3"], it * NMM3 + 8)
                    if it > 0:
                        w.wait(sems["dmo7"], it * 16)
                    vector.tensor_copy(
                        out=o_sb[:, 7, a:b], in_=ps[7][:, 0:b - a],
                    ).then_inc(sems["cp3t"])


            @block.gpsimd
            def _(gpsimd: bass.BassEngine):
                w = _Waiter(gpsimd)
                gpsimd.memset(ath_sb[:], 0).then_inc(sems["ath0"])
                for it in range(iters):
                    for m in range(G3 - 1):
                        if m % 2 == 0:
                            w.wait(sems["cp3v"], it * 4 + m // 2 + 1)
                        else:
                            w.wait(sems["cp3s"], it * 3 + m // 2 + 1)
                        if it > 0:
                            w.wait(sems[f"dmo{m}"], it * 16)
                        gpsimd.dma_start(
                            out=out[128 * m:128 * m + 128, :],
                            in_=o_sb[:, m, :],
                        ).then_inc(sems[f"dmo{m}"], 16)
                    a, b = M7SUB[0]
                    w.wait(sems["cp3t"], it + 1)
                    if it > 0:
                        w.wait(sems["dmo7"], it * 16)
                    gpsimd.dma_start(
                        out=out[896:1024, a:b],
                        in_=o_sb[:, 7, a:b],
                    ).then_inc(sems["dmo7"], 16)
                for m in range(G3 - 1):
                    gpsimd.wait_ge(sems[f"dmo{m}"], iters * 16)
                gpsimd.wait_ge(sems["dmo7"], iters * 16)
                gpsimd.wait_ge(sems["dmo7b"], iters * 16)

            @block.scalar
            def _(scalar: bass.BassEngine):
                w = _Waiter(scalar)
                for it in range(iters):
                    # wave A copies lt3,4
                    for lt in (3, 4):
                        w.wait(sems["mmA"], it * LT + lt + 1)
                        scalar.copy(v_sb[:, lt, 0:512], ps[lt][:, :]
                                    ).then_inc(sems["cpA_s"])
                    # wave B copies + at_hi even, interleaved by readiness
                    BSEQ = [("hi", 0), ("hi", 2), ("vB", 2),
                            ("lo", 0), ("vB", 3), ("lo", 2),
                            ("hi", 4), ("hi", 6), ("lo", 4), ("lo", 6)]
                    for kind, i in BSEQ:
                        if kind == "vB":
                            w.wait(sems["mm1"], it * LT + i + 1)
                            bank = (ps[B1_BANKS[i]] if i < 3
                                    else ps[B2_BANKS[i - 3]])
                            scalar.copy(v_sb[:, i, 512:1024], bank[:, :]
                                        ).then_inc(sems["cpB"])
                        elif kind == "hi":
                            w.wait(sems["mm2"], it * G2 + i + 1)
                            if it == 0 and i == 0:
                                w.wait(sems["ath0"], 1)
                            scalar.copy(ath_sb[:, i, :],
                                        ps[PH2_BANKS[i % 4]][:, :]
                                        ).then_inc(sems["cp2h_a"])
                        else:
                            w.wait(sems["mm2b"], it * 4 + i // 2 + 1)
                            scalar.copy(atl_sb[:, i, :],
                                        ps[PH2_BANKS[i % 4]][:, :]
                                        ).then_inc(sems["cp2l_a"])
                    for m in (1, 3, 5):
                        w.wait(sems["mm3"], it * NMM3 + m + 1)
                        if it > 0:
                            w.wait(sems[f"dmo{m}"], it * 16)
                        scalar.copy(o_sb[:, m, :],
                                    ps[PH3_BANKS[m % 4]][:, :]
                                    ).then_inc(sems["cp3s"])
                    a, b = M7SUB[1]
                    w.wait(sems["mm3"], it * NMM3 + 9)
                    if it > 0:
                        w.wait(sems["dmo7b"], it * 16)
                    scalar.copy(o_sb[:, 7, a:b], ps[2][:, 0:b - a]
                                ).then_inc(sems["cp3u"])
                    scalar.wait_ge(sems["cp3u"], it + 1)
                    scalar.dma_start(out=out[896:1024, a:b],
                                     in_=o_sb[:, 7, a:b]
                                     ).then_inc(sems["dmo7b"], 16)

    return nc


# ---------------- host side ----------------

_GRAPH_CACHE: dict = {}


def get_graph(iters: int = 1, banded: bool = True) -> bass.Bass:
    key = (iters, banded)
    if key not in _GRAPH_CACHE:
        _GRAPH_CACHE[key] = build_graph(iters, banded)
    return _GRAPH_CACHE[key]


class Runner:
    """Compile-once executor for one Bass graph across the 8 cores.

    Mirrors bass2jax.run_bass_via_pjrt but keeps the jitted callable so
    repeated invocations don't re-trace/re-compile.
    """

    def __init__(self, nc: bass.Bass, n_cores: int = N_CORES):
        import jax
        from jax.sharding import Mesh, PartitionSpec
        from jax.experimental.shard_map import shard_map
        from concourse import bass2jax, mybir as _mb

        bass2jax.install_neuronx_cc_hook()
        self.n_cores = n_cores

        partition_name = (nc.partition_id_tensor.name
                          if nc.partition_id_tensor else None)
        in_names, out_names, out_avals, zero_shapes = [], [], [], []
        for alloc in nc.m.functions[0].allocations:
            if not isinstance(alloc, _mb.MemoryLocationSet):
                continue
            name = alloc.memorylocations[0].name
            if alloc.kind == "ExternalInput":
                if name != partition_name:
                    in_names.append(name)
            elif alloc.kind == "ExternalOutput":
                out_names.append(name)
                shape = tuple(alloc.tensor_shape)
                dtype = _mb.dt.np(alloc.dtype)
                out_avals.append(jax.core.ShapedArray(shape, dtype))
                zero_shapes.append((shape, dtype))
        self.in_names = list(in_names)
        self.out_names = out_names
        self.out_avals = out_avals
        self.zero_shapes = zero_shapes
        n_params = len(in_names)
        all_names = in_names + out_names
        if partition_name is not None:
            all_names = all_names + [partition_name]

        def _body(*args):
            operands = list(args)
            if partition_name is not None:
                operands.append(bass2jax.partition_id_tensor())
            outs = bass2jax._bass_exec_p.bind(
                *operands,
                out_avals=tuple(out_avals),
                in_names=tuple(all_names),
                out_names=tuple(out_names),
                lowering_input_output_aliases=(),
                sim_require_finite=True,
                sim_require_nnan=True,
                nc=nc,
            )
            return tuple(outs)

        devices = jax.devices()[:n_cores]
        mesh = Mesh(np.asarray(devices), ("core",))
        self._mesh = mesh
        n_outs = len(out_names)
        self._fn = jax.jit(
            shard_map(_body, mesh=mesh,
                      in_specs=(PartitionSpec("core"),) * (n_params + n_outs),
                      out_specs=(PartitionSpec("core"),) * n_outs,
                      check_rep=False),
            donate_argnums=tuple(range(n_params, n_params + n_outs)),
            keep_unused=True,
        )

    def stage(self, in_maps):
        """device_put the concatenated inputs once; returns device arrays."""
        import jax
        concat_in = [
            np.concatenate([np.asarray(m[name]) for m in in_maps], axis=0)
            for name in self.in_names
        ]
        return [jax.device_put(a) for a in concat_in]

    def make_zeros(self):
        if not hasattr(self, "_zeros_fn"):
            import jax
            import jax.numpy as jnp
            from jax.sharding import NamedSharding, PartitionSpec
            shardings = tuple(
                NamedSharding(self._mesh, PartitionSpec("core"))
                for _ in self.zero_shapes)
            shapes = [((self.n_cores * s[0], *s[1:]), d)
                      for s, d in self.zero_shapes]

            def _mk():
                return tuple(jnp.zeros(sh, dt) for sh, dt in shapes)

            self._zeros_fn = jax.jit(_mk, out_shardings=shardings)
        return list(self._zeros_fn())

    def run_staged(self, dev_in, dev_zeros):
        return self._fn(*dev_in, *dev_zeros)

    def __call__(self, in_maps):
        out_arrs = self._fn(*self.stage(in_maps), *self.make_zeros())
        return [
            {name: np.asarray(out_arrs[i]).reshape(
                self.n_cores, *self.out_avals[i].shape)[c]
             for i, name in enumerate(self.out_names)}
            for c in range(self.n_cores)
        ]


_RUNNER_CACHE: dict = {}


def get_runner(iters: int = 1) -> "Runner":
    if iters not in _RUNNER_CACHE:
        _RUNNER_CACHE[iters] = Runner(get_graph(iters))
    return _RUNNER_CACHE[iters]


def _fp8_split(a32: np.ndarray):
    """(hi, lo) fp8 e4m3 pair with hi + lo ~= a32 (already scaled)."""
    f8 = ml_dtypes.float8_e4m3
    hi = a32.astype(f8)
    lo = (a32 - hi.astype(np.float32)).astype(f8)
    return hi, lo


def _pack_pairs(a: np.ndarray) -> np.ndarray:
    """[1024, F] -> [128, KP, 2, F]: d -> (pair j, i, partition p)."""
    F = a.shape[1]
    return np.ascontiguousarray(
        a.reshape(KP, 2, 128, F).transpose(2, 0, 1, 3))


def make_in_maps(values: np.ndarray, input_weights: np.ndarray,
                 output_weight: np.ndarray) -> list:
    f8 = ml_dtypes.float8_e4m3
    w1h, w1l = _fp8_split(np.ascontiguousarray(input_weights.T) * SW1)
    w2h, w2l = _fp8_split(np.ascontiguousarray(output_weight.T) * SW2)
    # w1 layout [128, half, pair, hilo, i, n]
    w1_pack = np.empty((128, 2, KP, 2, 2, 512), dtype=f8)
    for half in range(2):
        w1_pack[:, half, :, 0] = _pack_pairs(w1h[:, 512 * half:512 * half + 512])
        w1_pack[:, half, :, 1] = _pack_pairs(w1l[:, 512 * half:512 * half + 512])
    w2_pack = np.empty((128, 2, KP, 2, KT, 128), dtype=f8)
    w2_pack[:, 0] = _pack_pairs(w2h).reshape(128, KP, 2, KT, 128)
    w2_pack[:, 1] = _pack_pairs(w2l).reshape(128, KP, 2, KT, 128)
    tpt = gauss_toeplitz_packed()
    nid = np.zeros((128, 2, 2, 128), dtype=f8)
    eye = -np.eye(128, dtype=np.float32)
    nid[:, 0, 0] = eye.astype(f8)
    nid[:, 1, 1] = eye.astype(f8)
    in_maps = []
    for core in range(N_CORES):
        b, c = divmod(core, 4)
        lo_r, hi_r = c * CHUNK - HALO_L, c * CHUNK + CHUNK + HALO_R
        src_lo, src_hi = max(lo_r, 0), min(hi_r, L)
        xt_pad = np.zeros((D, LPAD), dtype=np.float32)
        xt_pad[:, src_lo - lo_r:src_hi - lo_r] = \
            values[b, src_lo:src_hi, :].T * SX
        xh, xl = _fp8_split(xt_pad)
        x_pack = np.empty((128, KP, 2, 2, LPAD), dtype=f8)
        x_pack[:, :, 0] = _pack_pairs(xh)
        x_pack[:, :, 1] = _pack_pairs(xl)
        in_maps.append({"xhl": x_pack, "w1": w1_pack, "w2": w2_pack,
                        "tp": tpt, "nid": nid})
    return in_maps


def assemble(results: list) -> np.ndarray:
    out = np.empty((B, L, D), dtype=np.float32)
    for core in range(N_CORES):
        b, c = divmod(core, 4)
        out[b, c * CHUNK:(c + 1) * CHUNK, :] = \
            results[core]["out"].T.astype(np.float32) / OUT_DESCALE
    return out


def kernel(values: np.ndarray, input_weights: np.ndarray,
           output_weight: np.ndarray) -> np.ndarray:
    in_maps = make_in_maps(values, input_weights, output_weight)
    try:
        return assemble(get_runner(1)(in_maps))
    except Exception:
        # fallback: canonical SPMD path (re-traces per call but always works)
        res = run_bass_kernel_spmd(get_graph(1), in_maps,
                                   core_ids=list(range(N_CORES)))
        return assemble(res.results)


# revision 60
# speedup vs baseline: 1.2181x; 1.0026x over previous
"""Trainium2 Bass kernel for nn_Attention (Gaussian banded attention).

Math (reference):
    v = values @ input_weights.T                      # [B,L,D]
    probs[h,q,k] = N(k - q - off_h; std_h)            # Gaussian, depends on k-q only
    attended[b,h,q,:] = sum_k probs[h,q,k] v[b,k,h*pd:(h+1)*pd]
    out = attended_merged @ output_weight.T           # [B,L,D]

Key structural facts exploited:
  - probs is a banded Toeplitz matrix per head (6-sigma truncation), so
    attention is a narrow depthwise convolution along L; no [L,L] matmul.
  - Batch x L sharding is embarrassingly parallel given a halo of
    56 backward / 40 forward input rows (row-wise projection, no bias).
  - The two dense [1024,1024] projections run as fp8(e4m3) DoubleRow
    matmuls (0.5 cycles/row, 256-deep contraction = 4x bf16 FLOP rate),
    error-compensated with a 3-term split:
        x @ w ~= Xh@Wh + Xl@Wh + Xh@Wl,
    where Xh = fp8(x*s), Xl = fp8(x*s - Xh) (likewise W). Power-of-2
    scales keep everything in fp8's normal range and are folded into the
    host-prepared tables / final host descale, so no on-device rescaling
    is needed. Measured end-to-end error ~3e-3 (better than 2e-2 gate).
  - The banded attention itself stays bf16 (windowed Toeplitz matmuls;
    DoubleRow would widen the windows and lose the benefit).

Sharding: 8 cores = (B=2) x (4 chunks of 512 rows of L). No collectives.

Cost-model performance (CoreSim, TRN2 timing): 31,695 ns single execution
(vs 38,609 ns for the bf16 baseline); 26,329 ns/iter steady state
(vs 33,046). PE stream is gapless to within ~0.3us; single-shot additionally
pays ~2.4us of initial DMA latency and ~2.9us of output copy+DMA tail.
"""

import math
from contextlib import ExitStack

import numpy as np
import ml_dtypes

import concourse.bass as bass
from concourse import mybir
from concourse.bass_utils import run_bass_kernel_spmd

# ---- NEFF disk cache (keyed by BIR hash) to avoid recompiling identical
# graphs in fresh processes ----
import hashlib
import os
import shutil

_NEFF_CACHE_DIR = os.environ.get("NEFF_CACHE_DIR", "/root/neff_cache")


def _install_neff_cache():
    import concourse.bass_utils as _bu
    import concourse.bass2jax as _b2j
    if getattr(_bu, "_neff_cache_installed", False):
        return
    orig = _bu.compile_bir_kernel

    def cached(bir_json, tmpdir, neff_name="file.neff"):
        cpath = None
        try:
            os.makedirs(_NEFF_CACHE_DIR, exist_ok=True)
            key = hashlib.sha256(bir_json).hexdigest()[:32]
            cpath = os.path.join(_NEFF_CACHE_DIR, f"{key}.neff")
            dst = os.path.join(tmpdir, neff_name)
            if os.path.exists(cpath):
                shutil.copy(cpath, dst)
                return dst
        except OSError:
            cpath = None  # cache unusable; plain compile below
        path = orig(bir_json, tmpdir, neff_name)
        if cpath is not None:
            try:
                shutil.copy(path, cpath)
            except OSError:
                pass
        return path

    _bu.compile_bir_kernel = cached
    _b2j.compile_bir_kernel = cached
    _bu._neff_cache_installed = True


_install_neff_cache()

# ---------------- problem constants (hardcoded per spec) ----------------
B, L, D = 2, 2048, 1024
H, PD = 8, 128
ATTN_STD = np.array([1.0, 2.0, 4.0, 8.0, 1.0, 2.0, 4.0, 8.0], dtype=np.float64)
ATTN_OFFSET = np.array([-1.0, -2.0, -4.0, -8.0, -1.0, -2.0, -4.0, -8.0], dtype=np.float64)

N_CORES = 8
CHUNK = 512            # output rows per core
HALO_L, HALO_R = 56, 40
LPAD = 640             # 56 + 512 + 40 = 608, padded to 5*128
LT = 5                 # l-tiles of v (640 / 128)
KT = 8                 # d tiles (1024 / 128)
KP = 4                 # DoubleRow contraction pairs (1024 / 256)
NQ = CHUNK             # query columns per core

F8 = mybir.dt.float8e4
BF16 = mybir.dt.bfloat16
F32 = mybir.dt.float32
DR = mybir.MatmulPerfMode.DoubleRow

# power-of-2 scales for fp8 quantization; folded into host tables.
SX = 32.0              # x (values^T)
SW1 = 256.0            # input_weights^T
SW2 = 256.0            # output_weight^T
SA = 32.0              # attended (via tp table scale)
TP_SCALE = SA / (SX * SW1)     # folded into the Gaussian table
OUT_DESCALE = SA * SW2         # host divides the bf16 output by this

G2 = H                 # attention heads -> attendedT
G3 = KT                # proj2 d_out tiles -> outT
TP0, TPW = 408, 240    # tp column window (banded)


def gauss_toeplitz_packed() -> np.ndarray:
    """tp[p, h, c] = g_h(p - (TP0+c - 512) - 56) * TP_SCALE, [128, H, TPW] bf16.

    For v-tile t (rows k' = 128t + r of padded-local v) the attention rhs is
    tp[:, h, 512-128t+j0-TP0 : ...] so that rhs[r, q'] = g_h(128t + r - q' - 56)
    = probs[h, q, k].T in padded-local coordinates (scaled).
    """
    r = np.arange(128, dtype=np.float64)[:, None]
    m = TP0 + np.arange(TPW, dtype=np.float64)[None, :]
    delta = r - (m - 512.0) - 56.0  # = k - q
    tables = []
    for h in range(H):
        std, off = ATTN_STD[h], ATTN_OFFSET[h]
        z = (delta - off) / std
        g = np.exp(-0.5 * z * z) / (std * math.sqrt(2.0 * math.pi))
        g[np.abs(z) > 4.25] = 0.0
        tables.append(g * TP_SCALE)
    return np.stack(tables, axis=1).astype(ml_dtypes.bfloat16)  # [128, H, TPW]


def attn_windows(h: int):
    """Static (t, j0, j1) list: nonzero q-column window of v-tile t for head h,
    8-aligned. Coverage of [0,512) is guaranteed (window width > 128)."""
    std, off = int(ATTN_STD[h]), int(ATTN_OFFSET[h])
    wlo = -56 - off - int(4.25 * std)
    whi = 71 - off + int(4.25 * std)
    res = []
    for t in range(LT):
        j0 = max(0, 128 * t + wlo)
        j1 = min(NQ, 128 * t + whi + 1)
        if j0 >= j1:
            continue
        res.append((t, j0, j1))
    return res


class _Waiter:
    """Per-engine wait_ge deduplication: skip waits dominated by an earlier
    wait on the same semaphore (counts are monotone)."""

    def __init__(self, eng):
        self.eng = eng
        self.seen = {}

    def wait(self, sem, count):
        if self.seen.get(id(sem), -1) >= count:
            return
        self.seen[id(sem)] = count
        self.eng.wait_ge(sem, count)


def build_graph(iters: int = 1, banded: bool = True) -> bass.Bass:
    """One SPMD core program. iters>1 repeats the kernel (x/w1 DMAs
    re-issued; tp/w2 loaded once) with increasing semaphore thresholds.

    PE program order per iteration (single stream, no warmup needed --
    the cost-model p-state ramp expires by the time data arrives):
      wave A : v[:, 0:512] = x @ W1a, pair-outer, banks 0-4, fp8 DR x3 terms
      wave B1: v[:, 512:1024] lt=0,1,2 pair-outer, banks 5,6,7
      ph2 h0,h1,h2 (banks 2,3,4)  -- interleaved: only need wave-A copies
      wave B2: lt=3,4 pair-inner, banks 0,1
      ph2 h3 (bank 5), ph2 h4-7 (banks 2-5)
      ph3 m0-7 (banks 0,1,6,7), terms (w2h,ath),(w2l,ath),(w2h,atl);
            m=7 split into q-column subtiles for a short tail.
    Copies: DVE: waveA lt0-2, at hi/lo odd heads, out even m.
            Act: waveA lt3-4, waveB all, at_hi even heads, out odd m.
            Pool: at_lo even heads, out DMAs.
    """
    nc = bass.Bass()

    xhl = nc.declare_dram_parameter("xhl", [128, KP, 2, 2, LPAD], F8, isOutput=False)
    w1 = nc.declare_dram_parameter("w1", [128, 2, KP, 2, 2, 512], F8, isOutput=False)
    w2 = nc.declare_dram_parameter("w2", [128, 2, KP, 2, KT, 128], F8, isOutput=False)
    tp = nc.declare_dram_parameter("tp", [128, H, TPW], BF16, isOutput=False)
    nid = nc.declare_dram_parameter("nid", [128, 2, 2, 128], F8, isOutput=False)
    out = nc.declare_dram_parameter("out", [D, NQ], BF16, isOutput=True)

    M7SUB = [(0, 352), (352, 512)]     # q-column subtiles of the last group
    NMM3 = G3 + 1                      # mm3 increments per iteration

    with ExitStack() as ctx:
        e = ctx.enter_context
        xhl_sb = e(nc.sbuf_tensor("xhl_sb", [128, 2, KP, 2, 2, LPAD], F8))
        w1_sb = e(nc.sbuf_tensor("w1_sb", [128, 2, 2, KP, 2, 2, 512], F8))
        w2_sb = e(nc.sbuf_tensor("w2_sb", [128, 2, KP, 2, KT, 128], F8))
        tp_sb = e(nc.sbuf_tensor("tp_sb", [128, H, TPW], BF16))
        nid_sb = e(nc.sbuf_tensor("nid_sb", [128, 2, 2, 128], F8))
        v_sb = e(nc.sbuf_tensor("v_sb", [128, LT, D], BF16))
        ath_sb = e(nc.sbuf_tensor("ath_sb", [128, H, NQ], F8))
        atl_sb = e(nc.sbuf_tensor("atl_sb", [128, H, NQ], F8))
        o_sb = e(nc.sbuf_tensor("o_sb", [128, KT, NQ], BF16))
        ps = [e(nc.psum_tensor(f"ps{i}", [128, 512], F32)) for i in range(8)]

        sem_names = (["mmA", "mm1", "mm2", "mm3", "tp_a", "tp_b",
                      "cpA_v", "cpA_s", "cpB", "cpB_v",
                      "cp2h_a", "cp2h_v", "cp2l_a", "cp2l_v",
                      "mm2b", "nid_d", "ath0",
                      "cp3v", "cp3s", "cp3t", "cpB4_v",
                      "w2h1", "w2h2", "w2l1", "w2l2"]
                     + [f"x_d{j}b{p}" for j in range(KP) for p in (0, 1)]
                     + [f"wa_d{j}b{p}" for j in range(KP) for p in (0, 1)]
                     + [f"wb_d{j}b{p}" for j in range(KP) for p in (0, 1)]
                     + [f"x_l0b{p}" for p in (0, 1)]
                     + [f"wa_l0b{p}" for p in (0, 1)]
                     + [f"dmo{m}" for m in range(G3)]
                     + ["dmo7b", "cp3u"])
        sems = {n: e(nc.semaphore(n)) for n in sem_names}

        B1_BANKS = [5, 6, 7]           # wave B lt=0,1,2
        B2_BANKS = [0, 1]              # wave B lt=3,4
        PH2_BANKS = [2, 3, 4, 5]
        PH3_BANKS = [0, 1, 6, 7]
        TERMS1 = ((0, 0), (1, 0), (0, 1))   # (t=x hilo, u=w hilo)

        def vtile_wait(w, it, t, h):
            """wait for v_sb tile t for head h's column slice."""
            nhalf = h // 4
            if nhalf == 0:
                if t < 3:
                    w.wait(sems["cpA_v"], it * 3 + t + 1)
                else:
                    w.wait(sems["cpA_s"], it * 2 + t - 2)
            elif t < 2:
                w.wait(sems["cpB_v"], it * 2 + t + 1)
            elif t < 4:
                w.wait(sems["cpB"], it * 2 + t - 1)
            elif h < 6:
                w.wait(sems["cpB4_v"], it * 2 + 1)
            else:
                w.wait(sems["cpB4_v"], it * 2 + 2)

        def ph2_head(tensor, w, it, h, parts=(0, LT)):
            bank = ps[PH2_BANKS[h % 4]]
            # bank WAR: banks 2,3,4 <- wave A lt2,3,4 copies; bank 5 <- wave
            # B lt0 copy; h>=4 <- at_lo of head h-4
            if h == 0:
                w.wait(sems["cpA_v"], it * 3 + 3)
            elif h == 1:
                w.wait(sems["cpA_s"], it * 2 + 1)
            elif h == 2:
                w.wait(sems["cpA_s"], it * 2 + 2)
            elif h == 3:
                w.wait(sems["cpB_v"], it * 2 + 1)
            elif h == 4:
                w.wait(sems["cp2l_a"], it * 4 + 1)   # lo0
            elif h == 5:
                w.wait(sems["cp2l_v"], it * 4 + 1)   # lo1
            elif h == 6:
                w.wait(sems["cp2l_a"], it * 4 + 2)   # lo2
            else:
                w.wait(sems["cp2l_v"], it * 4 + 2)   # lo3
            if it == 0:
                w.wait(sems["tp_a" if h < 4 else "tp_b"], 16)
            ph2_windows(tensor, w, it, h, bank, *parts)

        def ph2_windows(tensor, w, it, h, bank, part0, part1):
            """Emit windows wi in [part0, part1); start on wi==0, stop+mm2
            on the last window overall."""
            windows = attn_windows(h) if banded else [
                (t, 0, NQ) for t in range(LT)]
            for wi in range(part0, min(part1, len(windows))):
                t, j0, j1 = windows[wi]
                vtile_wait(w, it, t, h)
                c0 = 512 - 128 * t + j0 - TP0
                c1 = 512 - 128 * t + j1 - TP0
                mm = tensor.matmul(
                    bank[:, j0:j1],
                    v_sb[:, t, 128 * h:128 * h + 128],
                    tp_sb[:, h, c0:c1],
                    start=(wi == 0), stop=(wi == len(windows) - 1),
                )
                if wi == len(windows) - 1:
                    mm.then_inc(sems["mm2"])

        def neg_ident(tensor, w, it, h):
            """psum[bank(h)] += (-I) @ ath[h] -> bank holds at_lo.
            Even heads only (their at_lo copy runs on Act, which cannot
            subtract); odd heads' DVE computes the residual directly."""
            assert h % 2 == 0
            w.wait(sems["cp2h_a"], it * 4 + h // 2 + 1)
            if it == 0 and h == 0:
                w.wait(sems["nid_d"], 16)
            tensor.matmul(
                ps[PH2_BANKS[h % 4]][:, :],
                nid_sb[:, 0, :, :],
                ath_sb[:, h:h + 1, :].to_broadcast([128, 2, NQ]),
                start=False, stop=True,
                perf_mode=DR,
                skip_group_check=True,
            ).then_inc(sems["mm2b"])

        with nc.Block() as block:

            @block.sync
            def _(sync: bass.BassEngine):

                for it in range(iters):
                    buf = it % 2
                    if it > 1:
                        # xhl/w1 buffer reuse: wave B (last reader) of it-2
                        sync.wait_ge(sems["mm1"], (it - 1) * LT)
                    # pair 0 split fine for a fast first matmul
                    sync.dma_start(out=xhl_sb[:, buf, 0, 0], in_=xhl[:, 0, 0]
                                   ).then_inc(sems[f"x_d0b{buf}"], 16)
                    sync.dma_start(out=w1_sb[:, buf, 0, 0, 0], in_=w1[:, 0, 0, 0]
                                   ).then_inc(sems[f"wa_d0b{buf}"], 16)
                    sync.dma_start(out=xhl_sb[:, buf, 0, 1], in_=xhl[:, 0, 1]
                                   ).then_inc(sems[f"x_l0b{buf}"], 16)
                    sync.dma_start(out=w1_sb[:, buf, 0, 0, 1], in_=w1[:, 0, 0, 1]
                                   ).then_inc(sems[f"wa_l0b{buf}"], 16)
                    for j in range(1, KP):
                        sync.dma_start(out=xhl_sb[:, buf, j], in_=xhl[:, j]
                                       ).then_inc(sems[f"x_d{j}b{buf}"], 16)
                        sync.dma_start(out=w1_sb[:, buf, 0, j], in_=w1[:, 0, j]
                                       ).then_inc(sems[f"wa_d{j}b{buf}"], 16)
                    for j in range(KP):
                        if it == 0 and j == 3:
                            sync.dma_start(out=tp_sb[:, 0:4], in_=tp[:, 0:4]
                                           ).then_inc(sems["tp_a"], 16)
                        sync.dma_start(out=w1_sb[:, buf, 1, j], in_=w1[:, 1, j]
                                       ).then_inc(sems[f"wb_d{j}b{buf}"], 16)
                    if it == 0:
                        sync.dma_start(out=nid_sb[:], in_=nid[:]
                                       ).then_inc(sems["nid_d"], 16)
                        sync.dma_start(out=tp_sb[:, 4:8], in_=tp[:, 4:8]
                                       ).then_inc(sems["tp_b"], 16)
                        sync.dma_start(out=w2_sb[:, 0, 0:2], in_=w2[:, 0, 0:2]
                                       ).then_inc(sems["w2h1"], 16)
                        sync.dma_start(out=w2_sb[:, 0, 2:4], in_=w2[:, 0, 2:4]
                                       ).then_inc(sems["w2h2"], 16)
                        sync.dma_start(out=w2_sb[:, 1, 0:2], in_=w2[:, 1, 0:2]
                                       ).then_inc(sems["w2l1"], 16)
                        sync.dma_start(out=w2_sb[:, 1, 2:4], in_=w2[:, 1, 2:4]
                                       ).then_inc(sems["w2l2"], 16)

            @block.tensor
            def _(tensor: bass.BassEngine):
                w = _Waiter(tensor)
                for it in range(iters):
                    buf = it % 2
                    nth = (it // 2 + 1) * 16  # per-parity DMA count

                    # ---- wave A: pair-outer, banks 0-4 ----
                    for j in range(KP):
                        for ti, (t, u) in enumerate(TERMS1):
                            if j == 0:
                                if ti == 0:
                                    w.wait(sems[f"x_d0b{buf}"], nth)
                                    w.wait(sems[f"wa_d0b{buf}"], nth)
                                elif ti == 1:
                                    w.wait(sems[f"x_l0b{buf}"], nth)
                                else:
                                    w.wait(sems[f"wa_l0b{buf}"], nth)
                            else:
                                w.wait(sems[f"x_d{j}b{buf}"], nth)
                                w.wait(sems[f"wa_d{j}b{buf}"], nth)
                            for lt in range(LT):
                                if j == 0 and ti == 0 and it > 0:
                                    # banks 0,1 <- ph3 m4,m5 copies (prev it);
                                    # banks 2,3,4 <- at_lo h4,h5,h6 (prev it)
                                    if lt == 0:
                                        w.wait(sems["cp3v"], (it - 1) * 4 + 3)
                                    elif lt == 1:
                                        w.wait(sems["cp3s"], (it - 1) * 3 + 3)
                                    elif lt == 2:
                                        w.wait(sems["cp3u"], it)
                                    elif lt == 3:
                                        w.wait(sems["cp2l_v"], (it - 1) * 4 + 3)
                                    else:
                                        w.wait(sems["cp2l_a"], (it - 1) * 4 + 4)
                                mm = tensor.matmul(
                                    ps[lt][:, :],
                                    xhl_sb[:, buf, j, t, :,
                                           128 * lt:128 * lt + 128],
                                    w1_sb[:, buf, 0, j, u, :, :],
                                    start=(j == 0 and ti == 0),
                                    stop=(j == KP - 1 and ti == 2),
                                    perf_mode=DR,
                                )
                                if j == KP - 1 and ti == 2:
                                    mm.then_inc(sems["mmA"])

                    # ---- wave B1: lt=0,1,2 pair-outer, banks 5,6,7 ----
                    for j in range(KP):
                        w.wait(sems[f"wb_d{j}b{buf}"], nth)
                        for ti, (t, u) in enumerate(TERMS1):
                            for li, lt in enumerate((0, 1, 2)):
                                if j == 0 and ti == 0 and it > 0:
                                    if lt == 0:   # bank5 <- at_lo h7 prev
                                        w.wait(sems["cp2l_v"], (it - 1) * 4 + 4)
                                    elif lt == 1:  # bank6 <- ph3 m6 prev
                                        w.wait(sems["cp3v"], (it - 1) * 4 + 4)
                                    else:          # bank7 <- ph3 m7a prev
                                        w.wait(sems["cp3t"], it)
                                mm = tensor.matmul(
                                    ps[B1_BANKS[li]][:, :],
                                    xhl_sb[:, buf, j, t, :,
                                           128 * lt:128 * lt + 128],
                                    w1_sb[:, buf, 1, j, u, :, :],
                                    start=(j == 0 and ti == 0),
                                    stop=(j == KP - 1 and ti == 2),
                                    perf_mode=DR,
                                )
                                if j == KP - 1 and ti == 2:
                                    mm.then_inc(sems["mm1"])

                    # ---- ph2 h0-h3 (banks 2,3,4,5; h3's bank free after
                    # ---- the vB0 copy) ----
                    for h in (0, 1, 2, 3):
                        ph2_head(tensor, w, it, h)
                    neg_ident(tensor, w, it, 0)

                    # ---- wave B2: lt=3,4 pair-inner, banks 0,1 ----
                    for li, lt in enumerate((3, 4)):
                        if lt == 4:
                            neg_ident(tensor, w, it, 2)
                        bank = ps[B2_BANKS[li]]
                        w.wait(sems["cpA_v"], it * 3 + li + 1)
                        for j in range(KP):
                            for ti, (t, u) in enumerate(TERMS1):
                                mm = tensor.matmul(
                                    bank[:, :],
                                    xhl_sb[:, buf, j, t, :,
                                           128 * lt:128 * lt + 128],
                                    w1_sb[:, buf, 1, j, u, :, :],
                                    start=(j == 0 and ti == 0),
                                    stop=(j == KP - 1 and ti == 2),
                                    perf_mode=DR,
                                )
                                if j == KP - 1 and ti == 2:
                                    mm.then_inc(sems["mm1"])

                    # ---- ph3: banks [0,1,6,7]; terms (w2h,ath),(w2l,ath),
                    # ---- (w2h,atl); m0+m1 interleaved around the late-head
                    # ---- at chains; m=7 split into q subtiles ----
                    TERMS3 = ((0, 0), (1, 0), (0, 1))   # (u, lo)

                    def ph3_mm(m, ti, j, bank, a, b, first, last):
                        u, lo = TERMS3[ti]
                        rhs_t = atl_sb if lo else ath_sb
                        if m <= 1:
                            if ti == 0:
                                w.wait(sems["cp2h_a"], it * 4 + j + 1)
                                w.wait(sems["cp2h_v"], it * 4 + j + 1)
                                if it == 0 and j == 0:
                                    w.wait(sems["w2h1"], 16)
                                if it == 0 and j == 2:
                                    w.wait(sems["w2h2"], 16)
                            elif ti == 1:
                                if it == 0 and j == 0:
                                    w.wait(sems["w2l1"], 16)
                                if it == 0 and j == 2:
                                    w.wait(sems["w2l2"], 16)
                            else:
                                w.wait(sems["cp2l_a"], it * 4 + j + 1)
                                w.wait(sems["cp2l_v"], it * 4 + j + 1)
                        mm = tensor.matmul(
                            bank[:, 0:b - a],
                            w2_sb[:, u, j, :, m, :],
                            rhs_t[:, 2 * j:2 * j + 2, a:b],
                            start=first, stop=last,
                            perf_mode=DR,
                        )
                        if last:
                            mm.then_inc(sems["mm3"])


                    # ---- ph2 h4-7; h4/h5 t=4 windows (which need the vB4
                    # ---- copy) deferred behind h5's earlier windows ----
                    ph2_head(tensor, w, it, 4, parts=(0, 4))
                    ph2_head(tensor, w, it, 5, parts=(0, 4))
                    ph2_head(tensor, w, it, 6, parts=(0, 3))
                    # fill the cpB4_v wait with m0's early matmuls
                    seen01 = {0: 0, 1: 0}
                    w.wait(sems["cpB"], it * 2 + 2)       # bank0 <- vB3
                    for (ti, j) in ((0, 0), (0, 1), (1, 0), (1, 1)):
                        seen01[0] += 1
                        ph3_mm(0, ti, j, ps[PH3_BANKS[0]], 0, NQ,
                               seen01[0] == 1, False)
                    ph2_windows(tensor, w, it, 4, ps[PH2_BANKS[0]], 4, 5)
                    ph2_windows(tensor, w, it, 5, ps[PH2_BANKS[1]], 4, 5)
                    ph2_windows(tensor, w, it, 6, ps[PH2_BANKS[2]], 3, 5)
                    ph2_head(tensor, w, it, 7)
                    neg_ident(tensor, w, it, 4)

                    # m0 (bank 0) + m1 (bank 1) interleaved; m0's first
                    # four matmuls already ran in the ph2 deferral window
                    SCHED01 = [(1, 0, 0), (1, 0, 1),
                               (1, 1, 0), (1, 1, 1),
                               ("ni", 6, 0),
                               (0, 2, 0), (0, 2, 1), (1, 2, 0), (1, 2, 1),
                               (0, 0, 2), (0, 1, 2), (1, 0, 2), (1, 1, 2),
                               (0, 0, 3), (0, 1, 3), (1, 0, 3), (1, 1, 3),
                               (0, 2, 2), (1, 2, 2), (0, 2, 3), (1, 2, 3)]
                    w.wait(sems["cpB4_v"], it * 2 + 2)    # bank1 <- vB4
                    total01 = {0: 12, 1: 12}
                    for (m, ti, j) in SCHED01:
                        if m == "ni":
                            neg_ident(tensor, w, it, ti)
                            continue
                        seen01[m] += 1
                        ph3_mm(m, ti, j, ps[PH3_BANKS[m]], 0, NQ,
                               seen01[m] == 1, seen01[m] == total01[m])

                    SEQ3 = ([(t, j) for j in (0, 1) for t in (0, 1, 2)]
                            + [(0, 2), (1, 2), (0, 3), (1, 3),
                               (2, 2), (2, 3)])
                    for m in range(2, G3):
                        bank = ps[PH3_BANKS[m % 4]]
                        if m == 2:
                            w.wait(sems["cpB_v"], it * 2 + 2)   # <- vB1
                        elif m == 3:
                            w.wait(sems["cpB"], it * 2 + 1)     # <- vB2
                        elif m % 2 == 0:
                            w.wait(sems["cp3v"], it * 4 + (m - 4) // 2 + 1)
                        else:
                            w.wait(sems["cp3s"], it * 3 + (m - 4) // 2 + 1)
                        subs = M7SUB if m == G3 - 1 else [(0, NQ)]
                        for si, (a, b) in enumerate(subs):
                            if m == G3 - 1:
                                bank = ps[7] if si == 0 else ps[2]
                                if si == 1:
                                    # bank2 WAR: at_lo h4 copy (this iter)
                                    w.wait(sems["cp2l_a"], it * 4 + 3)
                            for qi, (ti, j) in enumerate(SEQ3):
                                ph3_mm(m, ti, j, bank, a, b,
                                       qi == 0, qi == len(SEQ3) - 1)

            @block.vector
            def _(vector: bass.BassEngine):
                w = _Waiter(vector)
                for it in range(iters):
                    # wave A copies lt0,1,2
                    for lt in range(3):
                        w.wait(sems["mmA"], it * LT + lt + 1)
                        vector.tensor_copy(
                            out=v_sb[:, lt, 0:512], in_=ps[lt][:, :],
                        ).then_inc(sems["cpA_v"])
                    # wave B lt0, lt1 copies (DVE is idle here; Act is the
                    # saturated engine in this window)
                    for li in (0, 1):
                        w.wait(sems["mm1"], it * LT + li + 1)
                        vector.tensor_copy(
                            out=v_sb[:, li, 512:1024],
                            in_=ps[B1_BANKS[li]][:, :],
                        ).then_inc(sems["cpB_v"])

                    # at hi+lo for odd heads; vB4 copy between h3 and h5
                    def at_hi(h):
                        w.wait(sems["mm2"], it * G2 + h + 1)
                        if it == 0 and h == 1:
                            w.wait(sems["ath0"], 1)
                        vector.tensor_copy(
                            out=ath_sb[:, h, :], in_=ps[PH2_BANKS[h % 4]][:, :],
                        ).then_inc(sems["cp2h_v"])

                    def at_lo(h):
                        w.wait(sems["cp2h_v"], it * 4 + (h - 1) // 2 + 1)
                        vector.tensor_sub(
                            atl_sb[:, h, :], ps[PH2_BANKS[h % 4]][:, :],
                            ath_sb[:, h, :],
                        ).then_inc(sems["cp2l_v"])

                    at_hi(1); at_lo(1)
                    at_hi(3); at_lo(3)
                    w.wait(sems["mm1"], it * LT + 5)
                    vector.tensor_copy(
                        out=v_sb[:, 4, 512:768], in_=ps[1][:, 0:256],
                    ).then_inc(sems["cpB4_v"])
                    vector.tensor_copy(
                        out=v_sb[:, 4, 768:1024], in_=ps[1][:, 256:512],
                    ).then_inc(sems["cpB4_v"])
                    at_hi(5); at_hi(7); at_lo(5); at_lo(7)
                    for mi, m in enumerate((0, 2, 4, 6)):
                        w.wait(sems["mm3"], it * NMM3 + m + 1)
                        if it > 0:
                            w.wait(sems[f"dmo{m}"], it * 16)
                        vector.tensor_copy(
                            out=o_sb[:, m, :],
                            in_=ps[PH3_BANKS[m % 4]][:, :],
                        ).then_inc(sems["cp3v"])
                    a, b = M7SUB[0]
                    w.wait(sems["mm3"], it * NMM3 + 8)
                    if it > 0:
                        w.wait(sems["dmo7"], it * 16)
                    vector.tensor_copy(
                        out=o_sb[:, 7, a:b], in_=ps[7][:, 0:b - a],
                    ).then_inc(sems["cp3t"])


            @block.gpsimd
            def _(gpsimd: bass.BassEngine):
                w = _Waiter(gpsimd)
                gpsimd.memset(ath_sb[:], 0).then_inc(sems["ath0"])
                for it in range(iters):
                    for m in range(G3 - 1):
                        if m % 2 == 0:
                            w.wait(sems["cp3v"], it * 4 + m // 2 + 1)
                        else:
                            w.wait(sems["cp3s"], it * 3 + m // 2 + 1)
                        if it > 0:
                            w.wait(sems[f"dmo{m}"], it * 16)
                        gpsimd.dma_start(
                            out=out[128 * m:128 * m + 128, :],
                            in_=o_sb[:, m, :],
                        ).then_inc(sems[f"dmo{m}"], 16)
                    a, b = M7SUB[0]
                    w.wait(sems["cp3t"], it + 1)
                    if it > 0:
                        w.wait(sems["dmo7"], it * 16)
                    gpsimd.dma_start(
                        out=out[896:1024, a:b],
                        in_=o_sb[:, 7, a:b],
                    ).then_inc(sems["dmo7"], 16)
                for m in range(G3 - 1):
                    gpsimd.wait_ge(sems[f"dmo{m}"], iters * 16)
                gpsimd.wait_ge(sems["dmo7"], iters * 16)
                gpsimd.wait_ge(sems["dmo7b"], iters * 16)

            @block.scalar
            def _(scalar: bass.BassEngine):
                w = _Waiter(scalar)
                for it in range(iters):
                    # wave A copies lt3,4
                    for lt in (3, 4):
                        w.wait(sems["mmA"], it * LT + lt + 1)
                        scalar.copy(v_sb[:, lt, 0:512], ps[lt][:, :]
                                    ).then_inc(sems["cpA_s"])
                    # wave B copies + at_hi even, interleaved by readiness
                    BSEQ = [("hi", 0), ("hi", 2), ("vB", 2),
                            ("lo", 0), ("vB", 3), ("lo", 2),
                            ("hi", 4), ("hi", 6), ("lo", 4), ("lo", 6)]
                    for kind, i in BSEQ:
                        if kind == "vB":
                            w.wait(sems["mm1"], it * LT + i + 1)
                            bank = (ps[B1_BANKS[i]] if i < 3
                                    else ps[B2_BANKS[i - 3]])
                            scalar.copy(v_sb[:, i, 512:1024], bank[:, :]
                                        ).then_inc(sems["cpB"])
                        elif kind == "hi":
                            w.wait(sems["mm2"], it * G2 + i + 1)
                            if it == 0 and i == 0:
                                w.wait(sems["ath0"], 1)
                            scalar.copy(ath_sb[:, i, :],
                                        ps[PH2_BANKS[i % 4]][:, :]
                                        ).then_inc(sems["cp2h_a"])
                        else:
                            w.wait(sems["mm2b"], it * 4 + i // 2 + 1)
                            scalar.copy(atl_sb[:, i, :],
                                        ps[PH2_BANKS[i % 4]][:, :]
                                        ).then_inc(sems["cp2l_a"])
                    for m in (1, 3, 5):
                        w.wait(sems["mm3"], it * NMM3 + m + 1)
                        if it > 0:
                            w.wait(sems[f"dmo{m}"], it * 16)
                        scalar.copy(o_sb[:, m, :],
                                    ps[PH3_BANKS[m % 4]][:, :]
                                    ).then_inc(sems["cp3s"])
                    a, b = M7SUB[1]
                    w.wait(sems["mm3"], it * NMM3 + 9)
                    if it > 0:
                        w.wait(sems["dmo7b"], it * 16)
                    scalar.copy(o_sb[:, 7, a:b], ps[2][:, 0:b - a]
                                ).then_inc(sems["cp3u"])
                    scalar.wait_ge(sems["cp3u"], it + 1)
                    scalar.dma_start(out=out[896:1024, a:b],
                                     in_=o_sb[:, 7, a:b]
                                     ).then_inc(sems["dmo7b"], 16)

    return nc


# ---------------- host side ----------------

_GRAPH_CACHE: dict = {}


def get_graph(iters: int = 1, banded: bool = True) -> bass.Bass:
    key = (iters, banded)
    if key not in _GRAPH_CACHE:
        _GRAPH_CACHE[key] = build_graph(iters, banded)
    return _GRAPH_CACHE[key]


class Runner:
    """Compile-once executor for one Bass graph across the 8 cores.

    Mirrors bass2jax.run_bass_via_pjrt but keeps the jitted callable so
    repeated invocations don't re-trace/re-compile.
    """

    def __init__(self, nc: bass.Bass, n_cores: int = N_CORES):
        import jax
        from jax.sharding import Mesh, PartitionSpec
        from jax.experimental.shard_map import shard_map
        from concourse import bass2jax, mybir as _mb

        bass2jax.install_neuronx_cc_hook()
        self.n_cores = n_cores

        partition_name = (nc.partition_id_tensor.name
                          if nc.partition_id_tensor else None)
        in_names, out_names, out_avals, zero_shapes = [], [], [], []
        for alloc in nc.m.functions[0].allocations:
            if not isinstance(alloc, _mb.MemoryLocationSet):
                continue
            name = alloc.memorylocations[0].name
            if alloc.kind == "ExternalInput":
                if name != partition_name:
                    in_names.append(name)
            elif alloc.kind == "ExternalOutput":
                out_names.append(name)
                shape = tuple(alloc.tensor_shape)
                dtype = _mb.dt.np(alloc.dtype)
                out_avals.append(jax.core.ShapedArray(shape, dtype))
                zero_shapes.append((shape, dtype))
        self.in_names = list(in_names)
        self.out_names = out_names
        self.out_avals = out_avals
        self.zero_shapes = zero_shapes
        n_params = len(in_names)
        all_names = in_names + out_names
        if partition_name is not None:
            all_names = all_names + [partition_name]

        def _body(*args):
            operands = list(args)
            if partition_name is not None:
                operands.append(bass2jax.partition_id_tensor())
            outs = bass2jax._bass_exec_p.bind(
                *operands,
                out_avals=tuple(out_avals),
                in_names=tuple(all_names),
                out_names=tuple(out_names),
                lowering_input_output_aliases=(),
                sim_require_finite=True,
                sim_require_nnan=True,
                nc=nc,
            )
            return tuple(outs)

        devices = jax.devices()[:n_cores]
        mesh = Mesh(np.asarray(devices), ("core",))
        self._mesh = mesh
        n_outs = len(out_names)
        self._fn = jax.jit(
            shard_map(_body, mesh=mesh,
                      in_specs=(PartitionSpec("core"),) * (n_params + n_outs),
                      out_specs=(PartitionSpec("core"),) * n_outs,
                      check_rep=False),
            donate_argnums=tuple(range(n_params, n_params + n_outs)),
            keep_unused=True,
        )

    def stage(self, in_maps):
        """device_put the concatenated inputs once; returns device arrays."""
        import jax
        concat_in = [
            np.concatenate([np.asarray(m[name]) for m in in_maps], axis=0)
            for name in self.in_names
        ]
        return [jax.device_put(a) for a in concat_in]

    def make_zeros(self):
        if not hasattr(self, "_zeros_fn"):
            import jax
            import jax.numpy as jnp
            from jax.sharding import NamedSharding, PartitionSpec
            shardings = tuple(
                NamedSharding(self._mesh, PartitionSpec("core"))
                for _ in self.zero_shapes)
            shapes = [((self.n_cores * s[0], *s[1:]), d)
                      for s, d in self.zero_shapes]

            def _mk():
                return tuple(jnp.zeros(sh, dt) for sh, dt in shapes)

            self._zeros_fn = jax.jit(_mk, out_shardings=shardings)
        return list(self._zeros_fn())

    def run_staged(self, dev_in, dev_zeros):
        return self._fn(*dev_in, *dev_zeros)

    def __call__(self, in_maps):
        out_arrs = self._fn(*self.stage(in_maps), *self.make_zeros())
        return [
            {name: np.asarray(out_arrs[i]).reshape(
                self.n_cores, *self.out_avals[i].shape)[c]
             for i, name in enumerate(self.out_names)}
            for c in range(self.n_cores)
        ]


_RUNNER_CACHE: dict = {}


def get_runner(iters: int = 1) -> "Runner":
    if iters not in _RUNNER_CACHE:
        _RUNNER_CACHE[iters] = Runner(get_graph(iters))
    return _RUNNER_CACHE[iters]


def _fp8_split(a32: np.ndarray):
    """(hi, lo) fp8 e4m3 pair with hi + lo ~= a32 (already scaled)."""
    f8 = ml_dtypes.float8_e4m3
    hi = a32.astype(f8)
    lo = (a32 - hi.astype(np.float32)).astype(f8)
    return hi, lo


def _pack_pairs(a: np.ndarray) -> np.ndarray:
    """[1024, F] -> [128, KP, 2, F]: d -> (pair j, i, partition p)."""
    F = a.shape[1]
    return np.ascontiguousarray(
        a.reshape(KP, 2, 128, F).transpose(2, 0, 1, 3))


def make_in_maps(values: np.ndarray, input_weights: np.ndarray,
                 output_weight: np.ndarray) -> list:
    f8 = ml_dtypes.float8_e4m3
    w1h, w1l = _fp8_split(np.ascontiguousarray(input_weights.T) * SW1)
    w2h, w2l = _fp8_split(np.ascontiguousarray(output_weight.T) * SW2)
    # w1 layout [128, half, pair, hilo, i, n]
    w1_pack = np.empty((128, 2, KP, 2, 2, 512), dtype=f8)
    for half in range(2):
        w1_pack[:, half, :, 0] = _pack_pairs(w1h[:, 512 * half:512 * half + 512])
        w1_pack[:, half, :, 1] = _pack_pairs(w1l[:, 512 * half:512 * half + 512])
    w2_pack = np.empty((128, 2, KP, 2, KT, 128), dtype=f8)
    w2_pack[:, 0] = _pack_pairs(w2h).reshape(128, KP, 2, KT, 128)
    w2_pack[:, 1] = _pack_pairs(w2l).reshape(128, KP, 2, KT, 128)
    tpt = gauss_toeplitz_packed()
    nid = np.zeros((128, 2, 2, 128), dtype=f8)
    eye = -np.eye(128, dtype=np.float32)
    nid[:, 0, 0] = eye.astype(f8)
    nid[:, 1, 1] = eye.astype(f8)
    in_maps = []
    for core in range(N_CORES):
        b, c = divmod(core, 4)
        lo_r, hi_r = c * CHUNK - HALO_L, c * CHUNK + CHUNK + HALO_R
        src_lo, src_hi = max(lo_r, 0), min(hi_r, L)
        xt_pad = np.zeros((D, LPAD), dtype=np.float32)
        xt_pad[:, src_lo - lo_r:src_hi - lo_r] = \
            values[b, src_lo:src_hi, :].T * SX
        xh, xl = _fp8_split(xt_pad)
        x_pack = np.empty((128, KP, 2, 2, LPAD), dtype=f8)
        x_pack[:, :, 0] = _pack_pairs(xh)
        x_pack[:, :, 1] = _pack_pairs(xl)
        in_maps.append({"xhl": x_pack, "w1": w1_pack, "w2": w2_pack,
                        "tp": tpt, "nid": nid})
    return in_maps


def assemble(results: list) -> np.ndarray:
    out = np.empty((B, L, D), dtype=np.float32)
    for core in range(N_CORES):
        b, c = divmod(core, 4)
        out[b, c * CHUNK:(c + 1) * CHUNK, :] = \
            results[core]["out"].T.astype(np.float32) / OUT_DESCALE
    return out


def kernel(values: np.ndarray, input_weights: np.ndarray,
           output_weight: np.ndarray) -> np.ndarray:
    in_maps = make_in_maps(values, input_weights, output_weight)
    try:
        return assemble(get_runner(1)(in_maps))
    except Exception:
        # fallback: canonical SPMD path (re-traces per call but always works)
        res = run_bass_kernel_spmd(get_graph(1), in_maps,
                                   core_ids=list(range(N_CORES)))
        return assemble(res.results)
